# revision 29
# baseline (speedup 1.0000x reference)
"""2-layer GAT (gnn_message_passing) on 8 TRN2 NeuronCores.

Strategy (graph/data parallel, per sharding hint):
  - Nodes are partitioned across 8 ranks (6250 dst nodes each). Each rank owns
    the segment-softmax + aggregation for its destination nodes.
  - Per layer, every rank computes the projected features (h = x @ W,
    attention source/dest logits al/ar fused into the same matmul via an
    augmented RHS) for ITS OWN nodes, writes them as rows of a gather table
    (768B rows for layer 1: 256 bf16 h + 8 f32 al; 256B rows for layer 2),
    then an AllGather replicates the full table to every rank.
  - Edge stage: destinations are degree-sorted and packed into tiles of 128
    (dst on partitions); each dst gets a padded run of incoming-edge "slots"
    along the free dimension. Source rows are fetched with dma_gather
    (SWDGE indexed gather). Since gather indices are int16, the table is
    addressed through two base windows (rows [0,32768) and
    [TROWS-32768, TROWS)), and each dst's slots are split into a "lo" and
    "hi" range accordingly. Tiles are greedily grouped (up to SLOT_CAP
    slots) into shared gather calls, and the lo/hi gathers of consecutive
    groups rotate over 4 SWDGE queues so their drains overlap.
  - Slot-grid math per (tile, region): e = leakyrelu(al_src + ar_dst),
    p = exp(e) (no max-subtract needed at these magnitudes), denom =
    free-dim reduce, normalization applied AFTER aggregation.
  - Aggregation: msg = h_src * p (p broadcast over channels via a step-0
    AP), then a pairwise tree of wide tensor adds along the slot dim.
  - Padding slots read a sentinel table row (h = 0, al = -1e30 -> p = 0).

The full output is assembled on the host from the 8 per-rank outputs
(undoing the degree-sort permutation).
"""

import sys
from contextlib import ExitStack
from dataclasses import dataclass

import numpy as np

for _p in ("/opt/trn_rl_repo",):
    if _p not in sys.path:
        sys.path.insert(0, _p)

import concourse.bass as bass
import concourse.bacc as bacc
import concourse.mybir as mybir
import concourse.tile as tile
from concourse import bass_utils

F32 = mybir.dt.float32
BF16 = mybir.dt.bfloat16
I16 = mybir.dt.int16
AL_SENT = -1.0e30
Alu = mybir.AluOpType
Act = mybir.ActivationFunctionType


@dataclass
class Cfg:
    N: int = 50000
    E: int = 500000          # edges before self-loops
    F_IN: int = 128
    HID: int = 32
    HEADS: int = 8
    OUT: int = 64
    NEG: float = 0.2
    R: int = 8
    SLOT_CAP: int = 32       # max slots per gather group (SBUF budget)
    hi_base: int = -1        # -1: auto (TROWS - 32768, clamped to >= 0)

    @property
    def HC1(self):
        return self.HEADS * self.HID     # 256

    @property
    def NPR(self):
        return self.N // self.R

    @property
    def CHUNK(self):
        return self.NPR + 1              # + sentinel row

    @property
    def TROWS(self):
        return self.R * self.CHUNK

    @property
    def T(self):
        return (self.NPR + 127) // 128   # dst tiles per rank

    @property
    def ROW1(self):
        return 384                       # bf16 elems: 256 h + 16 (8xf32 al) + pad

    @property
    def ROW2(self):
        return 128                       # bf16 elems: 64 h2 + 2 (1xf32 al2) + pad

    @property
    def HI_BASE(self):
        if self.hi_base >= 0:
            return self.hi_base
        return max(0, self.TROWS - 32768)

    @property
    def LO_LIM(self):
        # rows addressable from base 0
        return min(self.TROWS, 32768)


@dataclass
class Sched:
    perm: np.ndarray          # [R, NPR] perm[r][pos] = global node id
    sortpos: np.ndarray       # [N] position of node within its rank
    D_lo: np.ndarray          # [T]
    D_hi: np.ndarray          # [T]
    groups: list              # list of (t0, t1) tile ranges
    idx16: np.ndarray         # [R, 128, TOTCOL] int16
    call_cols: list           # per group: (lo_col0, lo_ncol, hi_col0, hi_ncol)
    sub_off: np.ndarray       # [T, 2] slot offset of (tile, region) in group buffer
    group_of: np.ndarray      # [T] group index of tile


def build_schedule(cfg: Cfg, src: np.ndarray, dst: np.ndarray) -> Sched:
    N, R, NPR, CHUNK, T = cfg.N, cfg.R, cfg.NPR, cfg.CHUNK, cfg.T
    deg = np.bincount(dst, minlength=N).astype(np.int64)
    odeg = np.bincount(src, minlength=N).astype(np.int64)

    # assign the highest out-degree nodes to the ranks whose table chunks sit
    # in the lo/hi window overlap, maximizing the flexible-edge fraction
    oorder = np.argsort(-odeg, kind="stable")
    fill_order = [3, 4, 2, 5, 1, 6, 0, 7]
    rank_of = np.empty(N, np.int64)
    for i, r in enumerate(fill_order):
        rank_of[oorder[i * NPR:(i + 1) * NPR]] = r

    sortpos = np.empty(N, np.int64)
    perm = np.empty((R, NPR), np.int64)
    for r in range(R):
        nodes = np.where(rank_of == r)[0]
        order = np.argsort(-deg[nodes], kind="stable")
        perm[r] = nodes[order]
        sortpos[perm[r]] = np.arange(NPR)
    # chunk row 0 of every rank is its sentinel row; real rows start at 1
    row_of = rank_of * CHUNK + 1 + sortpos        # [N] table row of each node

    src_row = row_of[src]
    # categories: 0 = forced lo, 1 = flexible, 2 = forced hi
    cat = np.where(src_row < cfg.HI_BASE, 0, np.where(src_row < cfg.LO_LIM, 1, 2))

    # global dst key: (rank, sorted position)
    dkey = rank_of[dst] * NPR + sortpos[dst]
    order = np.lexsort((cat, dkey))
    s_src_row = src_row[order]
    s_dkey = dkey[order]

    cnt = np.bincount(dkey, minlength=R * NPR)
    cnt_lo = np.bincount(dkey[cat == 0], minlength=R * NPR)
    cnt_hi = np.bincount(dkey[cat == 2], minlength=R * NPR)
    start = np.concatenate([[0], np.cumsum(cnt)])[:-1]

    # per-dst lo count: balance towards half, respecting forced counts
    half = (cnt + 1) // 2
    nlo = np.clip(half, cnt_lo, cnt - cnt_hi)
    nhi = cnt - nlo

    pos_in_dst = np.arange(len(order)) - start[s_dkey]
    is_lo = pos_in_dst < nlo[s_dkey]
    slot = np.where(is_lo, pos_in_dst, pos_in_dst - nlo[s_dkey])

    # slot grid shared by all ranks
    D_lo = np.zeros(T, np.int64)
    D_hi = np.zeros(T, np.int64)
    nlo_g = nlo.reshape(R, NPR)
    nhi_g = nhi.reshape(R, NPR)
    for t in range(T):
        sl = slice(t * 128, min((t + 1) * 128, NPR))
        D_lo[t] = max(1, nlo_g[:, sl].max())
        D_hi[t] = max(1, nhi_g[:, sl].max())
    assert (D_lo + D_hi).max() <= cfg.SLOT_CAP, (
        f"tile needs {(D_lo + D_hi).max()} slots > SLOT_CAP {cfg.SLOT_CAP}")

    # greedy grouping of tiles, capped at SLOT_CAP slots
    groups = []
    group_of = np.zeros(T, np.int64)
    t0 = 0
    while t0 < T:
        t1 = t0 + 1
        tot = D_lo[t0] + D_hi[t0]
        while t1 < T and tot + D_lo[t1] + D_hi[t1] <= cfg.SLOT_CAP:
            tot += D_lo[t1] + D_hi[t1]
            t1 += 1
        group_of[t0:t1] = len(groups)
        groups.append((t0, t1))
        t0 = t1

    # slot offsets of each (tile, region) within its group buffer:
    # [lo slots of t0 | lo t1 | ... | hi t0 | hi t1 | ...]
    sub_off = np.zeros((T, 2), np.int64)
    call_cols = []
    col = 0
    pos_base_lo = np.zeros(T, np.int64)
    pos_base_hi = np.zeros(T, np.int64)
    for (t0, t1) in groups:
        S_lo = int(D_lo[t0:t1].sum())
        S_hi = int(D_hi[t0:t1].sum())
        off = 0
        lo_col0 = col
        for t in range(t0, t1):
            sub_off[t, 0] = off
            pos_base_lo[t] = col * 16 + off * 128
            off += D_lo[t]
        col += S_lo * 8  # 128/16 columns per slot-column
        hi_col0 = col
        off2 = 0
        for t in range(t0, t1):
            sub_off[t, 1] = S_lo + off2
            pos_base_hi[t] = col * 16 + off2 * 128
            off2 += D_hi[t]
        col += S_hi * 8
        call_cols.append((lo_col0, S_lo * 8, hi_col0, S_hi * 8))
    TOTCOL = col
    TOTPOS = TOTCOL * 16

    SENT_LO = 0                                    # rank 0 sentinel row
    SENT_HI = (R - 1) * CHUNK - cfg.HI_BASE        # last rank sentinel, local
    assert 0 <= SENT_HI < 32768

    # fill idx values per rank
    e_rank = s_dkey // NPR
    e_pos = s_dkey % NPR
    e_tile = e_pos // 128
    e_part = e_pos % 128
    idx16 = np.empty((R, 128, TOTCOL), np.int16)
    for r in range(R):
        vals = np.empty(TOTPOS, np.int32)
        for (t0, t1), (lc0, lnc, hc0, hnc) in zip(groups, call_cols):
            vals[lc0 * 16:(lc0 + lnc) * 16] = SENT_LO
            vals[hc0 * 16:(hc0 + hnc) * 16] = SENT_HI
        m = (e_rank == r)
        mlo = m & is_lo
        mhi = m & ~is_lo
        p_lo = pos_base_lo[e_tile[mlo]] + slot[mlo] * 128 + e_part[mlo]
        vals[p_lo] = s_src_row[mlo]
        p_hi = pos_base_hi[e_tile[mhi]] + slot[mhi] * 128 + e_part[mhi]
        vals[p_hi] = s_src_row[mhi] - cfg.HI_BASE
        assert vals.min() >= 0 and vals.max() < 32768
        idx16[r] = np.tile(vals.reshape(-1, 16).T, (8, 1))

    return Sched(perm=perm, sortpos=sortpos, D_lo=D_lo, D_hi=D_hi,
                 groups=groups, idx16=idx16, call_cols=call_cols,
                 sub_off=sub_off, group_of=group_of)


def _bc(ap, shape):
    """broadcast an AP to shape (step-0 dims)"""
    return ap.broadcast_to(list(shape))


def build_program(cfg: Cfg, sch: Sched):
    """Build the single SPMD Bass program. Returns nc."""
    nc = bacc.Bacc("TRN2", target_bir_lowering=False, debug=False,
                   num_devices=cfg.R, num_swdge_queues=4)
    T, NPR, CHUNK, TROWS = cfg.T, cfg.NPR, cfg.CHUNK, cfg.TROWS
    HC1, H, HID, OUT = cfg.HC1, cfg.HEADS, cfg.HID, cfg.OUT
    ROW1, ROW2 = cfg.ROW1, cfg.ROW2
    TOTCOL = sch.idx16.shape[2]
    NT = T * 128

    # ---- I/O ----
    xT = nc.dram_tensor("xT", [cfg.F_IN, NT], F32, kind="ExternalInput")
    idxs_d = nc.dram_tensor("idxs", [128, TOTCOL], I16, kind="ExternalInput")
    W1_d = nc.dram_tensor("W1", [cfg.F_IN, HC1], F32, kind="ExternalInput")
    W1T_d = nc.dram_tensor("W1T", [HC1, cfg.F_IN], F32, kind="ExternalInput")
    A1s_d = nc.dram_tensor("A1s", [HC1, H], F32, kind="ExternalInput")
    A1d_d = nc.dram_tensor("A1d", [HC1, H], F32, kind="ExternalInput")
    B1_d = nc.dram_tensor("B1rep", [128, HC1], F32, kind="ExternalInput")
    W2_d = nc.dram_tensor("W2", [HC1, OUT], F32, kind="ExternalInput")
    W2T_d = nc.dram_tensor("W2T", [OUT, HC1], F32, kind="ExternalInput")
    a2s_d = nc.dram_tensor("a2s", [OUT, 1], F32, kind="ExternalInput")
    a2d_d = nc.dram_tensor("a2d", [OUT, 1], F32, kind="ExternalInput")
    B2_d = nc.dram_tensor("B2rep", [128, OUT], F32, kind="ExternalInput")
    out_d = nc.dram_tensor("out", [NT, OUT], F32, kind="ExternalOutput")

    KC = HC1 // 128   # contraction chunks over HC1 (2)

    with tile.TileContext(nc) as tc, ExitStack() as ctx:
        dram = ctx.enter_context(tc.tile_pool(name="dram", bufs=1, space="DRAM"))
        const = ctx.enter_context(tc.tile_pool(name="const", bufs=1))
        psum = ctx.enter_context(tc.tile_pool(name="psum", bufs=2, space="PSUM"))

        # DRAM scratch
        chunk1 = dram.tile([CHUNK, ROW1], BF16)
        table1 = dram.tile([TROWS, ROW1], BF16, addr_space="Shared")
        chunk2 = dram.tile([CHUNK, ROW2], BF16)
        table2 = dram.tile([TROWS, ROW2], BF16, addr_space="Shared")
        h1d = dram.tile([NT, HC1], BF16)

        # ---- persistent constants ----
        idx_s = const.tile([128, TOTCOL], I16, tag="idx")
        nc.sync.dma_start(idx_s[:, :], idxs_d[:, :])
        RHS1 = const.tile([128, HC1 + 2 * H], F32, tag="rhs1")
        nc.sync.dma_start(RHS1[:, 0:HC1], W1_d[:, :])
        B1_s = const.tile([128, HC1], F32, tag="b1")
        nc.sync.dma_start(B1_s[:, :], B1_d[:, :])
        B2_s = const.tile([128, OUT], F32, tag="b2")
        nc.sync.dma_start(B2_s[:, :], B2_d[:, :])
        RHS2 = const.tile([128, KC, OUT + 2], BF16, tag="rhs2")
        nc.gpsimd.dma_start(RHS2[:, :, 0:OUT],
                            W2_d.ap().rearrange("(k p) c -> p k c", p=128))
        arL = const.tile([128, T, H], F32, tag="arL")
        nc.vector.memset(arL[:, :, :], 0.0)
        ar2L = const.tile([128, T, 1], F32, tag="ar2L")
        nc.vector.memset(ar2L[:, :, :], 0.0)

        # ================= phase 1: projection + table 1 ====================
        TS_T = (NPR + 1 + 127) // 128
        full_t = NPR // 128
        rem = NPR - full_t * 128
        with tc.tile_pool(name="ph1", bufs=1) as ph1:
            xT_s = ph1.tile([128, NT], F32, tag="xT")
            nc.sync.dma_start(xT_s[:, :], xT[:, :])
            W1T_s = ph1.tile([128, KC, 128], F32, tag="w1t")
            nc.sync.dma_start(W1T_s[:, :, :],
                              W1T_d.ap().rearrange("(k p) f -> p k f", p=128))
            A1s_s = ph1.tile([128, KC, H], F32, tag="a1s")
            nc.sync.dma_start(A1s_s[:, :, :],
                              A1s_d.ap().rearrange("(k p) h -> p k h", p=128))
            A1d_s = ph1.tile([128, KC, H], F32, tag="a1d")
            nc.sync.dma_start(A1d_s[:, :, :],
                              A1d_d.ap().rearrange("(k p) h -> p k h", p=128))
            W2T_s = ph1.tile([OUT, HC1], F32, tag="w2t")
            nc.sync.dma_start(W2T_s[:, :], W2T_d[:, :])
            a2s_s = ph1.tile([OUT, 1], F32, tag="a2s")
            nc.sync.dma_start(a2s_s[:, :], a2s_d[:, :])
            a2d_s = ph1.tile([OUT, 1], F32, tag="a2d")
            nc.sync.dma_start(a2d_s[:, :], a2d_d[:, :])

            # fold attention vectors into projection RHS
            for (dst_off, A_s) in ((HC1, A1s_s), (HC1 + H, A1d_s)):
                ps = psum.tile([128, H], F32, tag="wprep")
                for k in range(KC):
                    nc.tensor.matmul(ps[:, :], W1T_s[:, k, :], A_s[:, k, :],
                                     start=(k == 0), stop=(k == KC - 1))
                nc.vector.tensor_copy(RHS1[:, dst_off:dst_off + H], ps[:, :])
            for (dst_off, a_s) in ((OUT, a2s_s), (OUT + 1, a2d_s)):
                for k in range(KC):
                    ps = psum.tile([128, 1], F32, tag="wprep2")
                    nc.tensor.matmul(ps[:, :], W2T_s[:, k * 128:(k + 1) * 128],
                                     a_s[:, :], start=True, stop=True)
                    nc.vector.tensor_copy(RHS2[:, k, dst_off:dst_off + 1],
                                          ps[:, :])

            tstage = ph1.tile([128, TS_T, ROW1], BF16, tag="tstage1")
            nc.vector.memset(tstage[:, :, :], 0.0)
            for t in range(T):
                ps = psum.tile([128, HC1 + 2 * H], F32, tag="proj1")
                nc.tensor.matmul(ps[:, :], xT_s[:, t * 128:(t + 1) * 128],
                                 RHS1[:, :], start=True, stop=True)
                nc.scalar.copy(tstage[:, t, 0:HC1], ps[:, 0:HC1])
                al_view = tstage[:, t, HC1:HC1 + 2 * H].bitcast(F32)
                nc.vector.tensor_copy(al_view[:, :], ps[:, HC1:HC1 + H])
                nc.vector.tensor_copy(arL[:, t, :],
                                      ps[:, HC1 + H:HC1 + 2 * H])
            # sentinel row -> chunk row 0 (h = 0, al = -1e30)
            sent1 = ph1.tile([1, ROW1], BF16, tag="sent1")
            nc.vector.memset(sent1[:, :], 0.0)
            nc.vector.memset(sent1[:, HC1:HC1 + 2 * H].bitcast(F32), AL_SENT)
            nc.sync.dma_start(chunk1[0:1, :], sent1[:, :])
            nc.sync.dma_start(
                chunk1[1:1 + full_t * 128, 0:HC1 + 2 * H].rearrange(
                    "(t p) c -> p t c", p=128),
                tstage[:, 0:full_t, 0:HC1 + 2 * H])
            if rem > 0:
                nc.sync.dma_start(chunk1[1 + full_t * 128:CHUNK, 0:HC1 + 2 * H],
                                  tstage[0:rem, full_t, 0:HC1 + 2 * H])
        nc.gpsimd.collective_compute(
            "AllGather", Alu.bypass,
            replica_groups=[list(range(cfg.R))],
            ins=[chunk1[:, :].opt()], outs=[table1[:, :].opt()])

        epool = ctx.enter_context(tc.tile_pool(name="edge", bufs=1))
        gpool = ctx.enter_context(tc.tile_pool(name="gpool", bufs=3))
        spool = ctx.enter_context(tc.tile_pool(name="spool", bufs=4))
        ppool = ctx.enter_context(tc.tile_pool(name="ppool", bufs=2))

        # ================= edge phase (shared for both layers) ==============
        def edge_layer(layer, table, ROW, CH, NH, arl_ap, out_cb):
            """layer: 1 or 2. CH: channels per head (32 / 64). NH: heads.
            arl_ap(t) -> [128, NH] f32 AP; out_cb(t, unn, rec) emits epilogue.
            """
            HCL = CH * NH
            lo_tab = table[0:cfg.LO_LIM, :]
            hi_tab = table[cfg.HI_BASE:TROWS, :]
            for gi, ((t0, t1), (lc0, lnc, hc0, hnc)) in enumerate(
                    zip(sch.groups, sch.call_cols)):
                S_lo = int(sch.D_lo[t0:t1].sum())
                S_hi = int(sch.D_hi[t0:t1].sum())
                S = S_lo + S_hi
                g = gpool.tile([128, cfg.SLOT_CAP, ROW], BF16, tag="gbuf")
                nc.gpsimd.dma_gather(
                    g[:, 0:S_lo, :], lo_tab, idx_s[:, lc0:lc0 + lnc],
                    num_idxs=S_lo * 128, num_idxs_reg=S_lo * 128,
                    elem_size=ROW, elem_step=ROW, single_packet=False,
                    queue_num=(2 * gi) % 4)
                nc.gpsimd.dma_gather(
                    g[:, S_lo:S, :], hi_tab, idx_s[:, hc0:hc0 + hnc],
                    num_idxs=S_hi * 128, num_idxs_reg=S_hi * 128,
                    elem_size=ROW, elem_step=ROW, single_packet=False,
                    queue_num=(2 * gi + 1) % 4)

                for t in range(t0, t1):
                    parts, dens = [], []
                    for reg in (0, 1):
                        D = int((sch.D_lo, sch.D_hi)[reg][t])
                        so = int(sch.sub_off[t, reg])
                        gs = g[:, so:so + D, :]
                        # e = leakyrelu(al + ar)
                        e = spool.tile([128, cfg.SLOT_CAP, NH], F32, tag="e")
                        al = gs[:, :, HCL:HCL + 2 * NH].bitcast(F32)
                        nc.vector.tensor_add(
                            e[:, 0:D, :], al,
                            _bc(arl_ap(t).unsqueeze(1), (128, D, NH)))
                        nc.vector.scalar_tensor_tensor(
                            e[:, 0:D, :], e[:, 0:D, :], cfg.NEG, e[:, 0:D, :],
                            op0=Alu.mult, op1=Alu.max)
                        # p = exp(e)  (bf16 out)
                        p = spool.tile([128, cfg.SLOT_CAP, NH], BF16, tag="p")
                        nc.scalar.activation(p[:, 0:D, :], e[:, 0:D, :],
                                             Act.Exp)
                        # denom partial = sum over slots
                        den = spool.tile([128, NH], F32, tag="den")
                        nc.vector.tensor_reduce(
                            den[:, :], p[:, 0:D, :].transpose([0, 2, 1]),
                            axis=mybir.AxisListType.X, op=Alu.add)
                        dens.append(den)
                        # msg = h * p (p broadcast over channels, step-0 AP)
                        msg = ppool.tile([128, cfg.SLOT_CAP, NH, CH], BF16,
                                         tag="msg")
                        pb = p[:, 0:D, :].unsqueeze(3).broadcast_to(
                            [128, D, NH, CH])
                        nc.vector.tensor_mul(
                            msg[:, 0:D, :, :],
                            gs[:, :, 0:HCL].rearrange("p d (h c) -> p d h c",
                                                      h=NH),
                            pb)
                        msg = msg[:, :, :, :].rearrange("p d h c -> p d (h c)")
                        # tree-sum over slots -> part [128, HCL] f32
                        part = spool.tile([128, HCL], F32, tag="part")
                        cur = D
                        while cur > 2:
                            hh = cur // 2
                            nc.vector.tensor_add(
                                msg[:, 0:hh, :], msg[:, 0:hh, :],
                                msg[:, cur - hh:cur, :])
                            cur -= hh
                        if cur == 2:
                            nc.vector.tensor_add(part[:, :], msg[:, 0, :],
                                                 msg[:, 1, :])
                        else:
                            nc.vector.tensor_copy(part[:, :], msg[:, 0, :])
                        parts.append(part)
                    unn = spool.tile([128, HCL], F32, tag="unn")
                    nc.vector.tensor_add(unn[:, :], parts[0][:, :],
                                         parts[1][:, :])
                    den = spool.tile([128, NH], F32, tag="dent")
                    nc.vector.tensor_add(den[:, :], dens[0][:, :],
                                         dens[1][:, :])
                    # den >= exp(leakyrelu(self-loop logit)) > 0: every dst
                    # has a self-loop and |e| is O(1), so no eps guard needed
                    rec = spool.tile([128, NH], F32, tag="rec")
                    nc.vector.reciprocal(rec[:, :], den[:, :])
                    out_cb(t, unn, rec)

        # ---- L1 epilogue: normalize, +b1, ELU, store h1 ----
        def l1_out(t, unn, rec):
            y = spool.tile([128, HC1], F32, tag="y1")
            nc.vector.tensor_mul(
                y.rearrange("p (h c) -> p h c", h=H),
                unn.rearrange("p (h c) -> p h c", h=H),
                _bc(rec[:, :].unsqueeze(2), (128, H, HID)))
            nc.vector.tensor_add(y[:, :], y[:, :], B1_s[:, :])
            mn = spool.tile([128, HC1], F32, tag="mn1")
            nc.vector.tensor_scalar_min(mn[:, :], y[:, :], 0.0)
            nc.vector.tensor_scalar_max(y[:, :], y[:, :], 0.0)
            em = spool.tile([128, HC1], F32, tag="em1")
            nc.scalar.activation(em[:, :], mn[:, :], Act.Exp)
            h1t = spool.tile([128, HC1], BF16, tag="h1t")
            nc.vector.scalar_tensor_tensor(h1t[:, :], em[:, :], -1.0, y[:, :],
                                           op0=Alu.add, op1=Alu.add)
            nc.sync.dma_start(h1d[t * 128:(t + 1) * 128, :], h1t[:, :])

        edge_layer(1, table1, ROW1, HID, H, lambda t: arL[:, t, :], l1_out)

        # ---- L2 projection from h1 (DMA-transpose h1d) ----
        h1T = epool.tile([128, KC, NT], BF16, tag="h1T")
        for k in range(KC):
            nc.sync.dma_start_transpose(h1T[:, k, :],
                                        h1d[:, k * 128:(k + 1) * 128])
        tstage2 = epool.tile([128, TS_T, ROW2], BF16, tag="tstage2")
        nc.vector.memset(tstage2[:, :, :], 0.0)
        for t in range(T):
            ps = psum.tile([128, OUT + 2], F32, tag="proj2")
            for k in range(KC):
                nc.tensor.matmul(ps[:, :], h1T[:, k, t * 128:(t + 1) * 128],
                                 RHS2[:, k, :], start=(k == 0),
                                 stop=(k == KC - 1))
            nc.scalar.copy(tstage2[:, t, 0:OUT], ps[:, 0:OUT])
            al2_view = tstage2[:, t, OUT:OUT + 2].bitcast(F32)
            nc.vector.tensor_copy(al2_view[:, :], ps[:, OUT:OUT + 1])
            nc.vector.tensor_copy(ar2L[:, t, :], ps[:, OUT + 1:OUT + 2])
        sent2 = epool.tile([1, ROW2], BF16, tag="sent2")
        nc.vector.memset(sent2[:, :], 0.0)
        nc.vector.memset(sent2[:, OUT:OUT + 2].bitcast(F32), AL_SENT)
        nc.sync.dma_start(chunk2[0:1, :], sent2[:, :])
        nc.sync.dma_start(
            chunk2[1:1 + full_t * 128, 0:OUT + 2].rearrange(
                "(t p) c -> p t c", p=128),
            tstage2[:, 0:full_t, 0:OUT + 2])
        if rem > 0:
            nc.sync.dma_start(chunk2[1 + full_t * 128:CHUNK, 0:OUT + 2],
                              tstage2[0:rem, full_t, 0:OUT + 2])
        nc.gpsimd.collective_compute(
            "AllGather", Alu.bypass,
            replica_groups=[list(range(cfg.R))],
            ins=[chunk2[:, :].opt()], outs=[table2[:, :].opt()])

        # ---- L2 epilogue: normalize, +b2, log_softmax, store out ----
        ostage = epool.tile([128, T, OUT], F32, tag="ostage")

        def l2_out(t, unn, rec):
            y = spool.tile([128, OUT], F32, tag="y2")
            nc.vector.tensor_scalar_mul(y[:, :], unn[:, :], rec[:, 0:1])
            nc.vector.tensor_add(y[:, :], y[:, :], B2_s[:, :])
            # |y| is O(10): exp stays finite in f32, no max-subtract needed
            ex = spool.tile([128, OUT], F32, tag="ex2")
            ssum = spool.tile([128, 1], F32, tag="ss2")
            nc.scalar.activation(ex[:, :], y[:, :], Act.Exp,
                                 accum_out=ssum[:, :])
            ls = spool.tile([128, 1], F32, tag="ls2")
            nc.scalar.activation(ls[:, :], ssum[:, :], Act.Ln)
            nc.vector.tensor_scalar_sub(ostage[:, t, :], y[:, :], ls[:, 0:1])

        edge_layer(2, table2, ROW2, OUT, 1, lambda t: ar2L[:, t, :], l2_out)
        nc.sync.dma_start(out_d.ap().rearrange("(t p) c -> p t c", p=128),
                          ostage[:, :, :])

    nc.compile()
    return nc


def _host_inputs(cfg: Cfg, sch: Sched, inputs: dict):
    """Build per-rank in_maps from the full problem inputs."""
    x = np.asarray(inputs["x"], np.float32)
    W1 = np.asarray(inputs["W1"], np.float32)
    a1_src = np.asarray(inputs["a1_src"], np.float32)
    a1_dst = np.asarray(inputs["a1_dst"], np.float32)
    b1 = np.asarray(inputs["b1"], np.float32)
    W2 = np.asarray(inputs["W2"], np.float32)
    a2_src = np.asarray(inputs["a2_src"], np.float32)
    a2_dst = np.asarray(inputs["a2_dst"], np.float32)
    b2 = np.asarray(inputs["b2"], np.float32)
    H, HID, HC1, OUT = cfg.HEADS, cfg.HID, cfg.HC1, cfg.OUT

    # block-diagonal per-head attention matrices: al = h @ A1s
    A1s = np.zeros((HC1, H), np.float32)
    A1d = np.zeros((HC1, H), np.float32)
    for h in range(H):
        A1s[h * HID:(h + 1) * HID, h] = a1_src[h]
        A1d[h * HID:(h + 1) * HID, h] = a1_dst[h]

    common = {
        "W1": np.ascontiguousarray(W1),
        "W1T": np.ascontiguousarray(W1.T),
        "A1s": A1s, "A1d": A1d,
        "B1rep": np.tile(b1[None, :], (128, 1)).astype(np.float32),
        "W2": np.ascontiguousarray(W2),
        "W2T": np.ascontiguousarray(W2.T),
        "a2s": np.ascontiguousarray(a2_src.reshape(OUT, 1)),
        "a2d": np.ascontiguousarray(a2_dst.reshape(OUT, 1)),
        "B2rep": np.tile(b2[None, :], (128, 1)).astype(np.float32),
    }
    in_maps = []
    for r in range(cfg.R):
        m = dict(common)
        xp = np.zeros((cfg.T * 128, x.shape[1]), np.float32)
        xp[:cfg.NPR] = x[sch.perm[r]]
        m["xT"] = np.ascontiguousarray(xp.T)
        m["idxs"] = np.ascontiguousarray(sch.idx16[r])
        in_maps.append(m)
    return in_maps


def run(cfg: Cfg, inputs: dict, trace: bool = False):
    edge_index = np.asarray(inputs["edge_index"])
    loops = np.arange(cfg.N, dtype=edge_index.dtype)
    src = np.concatenate([edge_index[0], loops]).astype(np.int64)
    dst = np.concatenate([edge_index[1], loops]).astype(np.int64)

    sch = build_schedule(cfg, src, dst)
    nc = build_program(cfg, sch)
    in_maps = _host_inputs(cfg, sch, inputs)
    res = bass_utils.run_bass_kernel_spmd(
        nc, in_maps, core_ids=list(range(cfg.R)), trace=trace)
    out = np.empty((cfg.N, cfg.OUT), np.float32)
    for r in range(cfg.R):
        o = res.results[r]["out"]
        out[sch.perm[r]] = o[:cfg.NPR]
    return out, res


def kernel(**inputs) -> np.ndarray:
    cfg = Cfg()
    out, _ = run(cfg, inputs)
    return out


if __name__ == "__main__":
    import reference
    inputs = {k: np.asarray(v) for k, v in reference.setup_inputs().items()}
    out = kernel(**inputs)
    exp = np.asarray(reference.reference(**reference.setup_inputs()))
    err = np.abs(out - exp).max() / (np.abs(exp).max() + 1e-12)
    print("rel err:", err)


# revision 32
# speedup vs baseline: 1.0169x; 1.0169x over previous
"""2-layer GAT (gnn_message_passing) on 8 TRN2 NeuronCores.

Strategy (graph/data parallel, per sharding hint):
  - Nodes are partitioned across 8 ranks (6250 dst nodes each). Each rank owns
    the segment-softmax + aggregation for its destination nodes.
  - Per layer, every rank computes the projected features (h = x @ W,
    attention source/dest logits al/ar fused into the same matmul via an
    augmented RHS) for ITS OWN nodes, writes them as rows of a gather table
    (768B rows for layer 1: 256 bf16 h + 8 f32 al; 256B rows for layer 2),
    then an AllGather replicates the full table to every rank.
  - Edge stage: destinations are degree-sorted and packed into tiles of 128
    (dst on partitions); each dst gets a padded run of incoming-edge "slots"
    along the free dimension. Source rows are fetched with dma_gather
    (SWDGE indexed gather). Since gather indices are int16, the table is
    addressed through two base windows (rows [0,32768) and
    [TROWS-32768, TROWS)), and each dst's slots are split into a "lo" and
    "hi" range accordingly. Tiles are greedily grouped (up to SLOT_CAP
    slots) into shared gather calls, and the lo/hi gathers of consecutive
    groups rotate over 4 SWDGE queues so their drains overlap.
  - Slot-grid math per (tile, region): e = leakyrelu(al_src + ar_dst),
    p = exp(e) (no max-subtract needed at these magnitudes), denom =
    free-dim reduce, normalization applied AFTER aggregation.
  - Aggregation: msg = h_src * p (p broadcast over channels via a step-0
    AP), then a pairwise tree of wide tensor adds along the slot dim.
  - Padding slots read a sentinel table row (h = 0, al = -1e30 -> p = 0).

The full output is assembled on the host from the 8 per-rank outputs
(undoing the degree-sort permutation).
"""

import sys
from contextlib import ExitStack
from dataclasses import dataclass

import numpy as np

for _p in ("/opt/trn_rl_repo",):
    if _p not in sys.path:
        sys.path.insert(0, _p)

import concourse.bass as bass
import concourse.bacc as bacc
import concourse.mybir as mybir
import concourse.tile as tile
from concourse import bass_utils

F32 = mybir.dt.float32
BF16 = mybir.dt.bfloat16
I16 = mybir.dt.int16
AL_SENT = -1.0e30
Alu = mybir.AluOpType
Act = mybir.ActivationFunctionType


@dataclass
class Cfg:
    N: int = 50000
    E: int = 500000          # edges before self-loops
    F_IN: int = 128
    HID: int = 32
    HEADS: int = 8
    OUT: int = 64
    NEG: float = 0.2
    R: int = 8
    SLOT_CAP: int = 36       # max slots per gather group (SBUF budget)
    hi_base: int = -1        # -1: auto (TROWS - 32768, clamped to >= 0)

    @property
    def HC1(self):
        return self.HEADS * self.HID     # 256

    @property
    def NPR(self):
        return self.N // self.R

    @property
    def CHUNK(self):
        return self.NPR + 1              # + sentinel row

    @property
    def TROWS(self):
        return self.R * self.CHUNK

    @property
    def T(self):
        return (self.NPR + 127) // 128   # dst tiles per rank

    @property
    def ROW1(self):
        return 384                       # bf16 elems: 256 h + 16 (8xf32 al) + pad

    @property
    def ROW2(self):
        return 128                       # bf16 elems: 64 h2 + 2 (1xf32 al2) + pad

    @property
    def HI_BASE(self):
        if self.hi_base >= 0:
            return self.hi_base
        return max(0, self.TROWS - 32768)

    @property
    def LO_LIM(self):
        # rows addressable from base 0
        return min(self.TROWS, 32768)


@dataclass
class Sched:
    perm: np.ndarray          # [R, NPR] perm[r][pos] = global node id
    sortpos: np.ndarray       # [N] position of node within its rank
    D_lo: np.ndarray          # [T]
    D_hi: np.ndarray          # [T]
    groups: list              # list of (t0, t1) tile ranges
    idx16: np.ndarray         # [R, 128, TOTCOL] int16
    call_cols: list           # per group: (lo_col0, lo_ncol, hi_col0, hi_ncol)
    sub_off: np.ndarray       # [T, 2] slot offset of (tile, region) in group buffer
    group_of: np.ndarray      # [T] group index of tile


def build_schedule(cfg: Cfg, src: np.ndarray, dst: np.ndarray) -> Sched:
    N, R, NPR, CHUNK, T = cfg.N, cfg.R, cfg.NPR, cfg.CHUNK, cfg.T
    deg = np.bincount(dst, minlength=N).astype(np.int64)
    odeg = np.bincount(src, minlength=N).astype(np.int64)

    # assign the highest out-degree nodes to the ranks whose table chunks sit
    # in the lo/hi window overlap, maximizing the flexible-edge fraction
    oorder = np.argsort(-odeg, kind="stable")
    fill_order = [3, 4, 2, 5, 1, 6, 0, 7]
    rank_of = np.empty(N, np.int64)
    for i, r in enumerate(fill_order):
        rank_of[oorder[i * NPR:(i + 1) * NPR]] = r

    sortpos = np.empty(N, np.int64)
    perm = np.empty((R, NPR), np.int64)
    for r in range(R):
        nodes = np.where(rank_of == r)[0]
        order = np.argsort(-deg[nodes], kind="stable")
        perm[r] = nodes[order]
        sortpos[perm[r]] = np.arange(NPR)
    # chunk row 0 of every rank is its sentinel row; real rows start at 1
    row_of = rank_of * CHUNK + 1 + sortpos        # [N] table row of each node

    src_row = row_of[src]
    # categories: 0 = forced lo, 1 = flexible, 2 = forced hi
    cat = np.where(src_row < cfg.HI_BASE, 0, np.where(src_row < cfg.LO_LIM, 1, 2))

    # global dst key: (rank, sorted position)
    dkey = rank_of[dst] * NPR + sortpos[dst]
    order = np.lexsort((cat, dkey))
    s_src_row = src_row[order]
    s_dkey = dkey[order]

    cnt = np.bincount(dkey, minlength=R * NPR)
    cnt_lo = np.bincount(dkey[cat == 0], minlength=R * NPR)
    cnt_hi = np.bincount(dkey[cat == 2], minlength=R * NPR)
    start = np.concatenate([[0], np.cumsum(cnt)])[:-1]

    # per-dst lo count: balance towards half, respecting forced counts
    half = (cnt + 1) // 2
    nlo = np.clip(half, cnt_lo, cnt - cnt_hi)
    nhi = cnt - nlo

    pos_in_dst = np.arange(len(order)) - start[s_dkey]
    is_lo = pos_in_dst < nlo[s_dkey]
    slot = np.where(is_lo, pos_in_dst, pos_in_dst - nlo[s_dkey])

    # slot grid shared by all ranks
    D_lo = np.zeros(T, np.int64)
    D_hi = np.zeros(T, np.int64)
    nlo_g = nlo.reshape(R, NPR)
    nhi_g = nhi.reshape(R, NPR)
    for t in range(T):
        sl = slice(t * 128, min((t + 1) * 128, NPR))
        D_lo[t] = max(1, nlo_g[:, sl].max())
        D_hi[t] = max(1, nhi_g[:, sl].max())
    assert (D_lo + D_hi).max() <= cfg.SLOT_CAP, (
        f"tile needs {(D_lo + D_hi).max()} slots > SLOT_CAP {cfg.SLOT_CAP}")

    # greedy grouping of tiles, capped at SLOT_CAP slots
    groups = []
    group_of = np.zeros(T, np.int64)
    t0 = 0
    while t0 < T:
        t1 = t0 + 1
        tot = D_lo[t0] + D_hi[t0]
        while t1 < T and tot + D_lo[t1] + D_hi[t1] <= cfg.SLOT_CAP:
            tot += D_lo[t1] + D_hi[t1]
            t1 += 1
        group_of[t0:t1] = len(groups)
        groups.append((t0, t1))
        t0 = t1

    # slot offsets of each (tile, region) within its group buffer:
    # [lo slots of t0 | lo t1 | ... | hi t0 | hi t1 | ...]
    sub_off = np.zeros((T, 2), np.int64)
    call_cols = []
    col = 0
    pos_base_lo = np.zeros(T, np.int64)
    pos_base_hi = np.zeros(T, np.int64)
    for (t0, t1) in groups:
        S_lo = int(D_lo[t0:t1].sum())
        S_hi = int(D_hi[t0:t1].sum())
        off = 0
        lo_col0 = col
        for t in range(t0, t1):
            sub_off[t, 0] = off
            pos_base_lo[t] = col * 16 + off * 128
            off += D_lo[t]
        col += S_lo * 8  # 128/16 columns per slot-column
        hi_col0 = col
        off2 = 0
        for t in range(t0, t1):
            sub_off[t, 1] = S_lo + off2
            pos_base_hi[t] = col * 16 + off2 * 128
            off2 += D_hi[t]
        col += S_hi * 8
        call_cols.append((lo_col0, S_lo * 8, hi_col0, S_hi * 8))
    TOTCOL = col
    TOTPOS = TOTCOL * 16

    SENT_LO = 0                                    # rank 0 sentinel row
    SENT_HI = (R - 1) * CHUNK - cfg.HI_BASE        # last rank sentinel, local
    assert 0 <= SENT_HI < 32768

    # fill idx values per rank
    e_rank = s_dkey // NPR
    e_pos = s_dkey % NPR
    e_tile = e_pos // 128
    e_part = e_pos % 128
    idx16 = np.empty((R, 128, TOTCOL), np.int16)
    for r in range(R):
        vals = np.empty(TOTPOS, np.int32)
        for (t0, t1), (lc0, lnc, hc0, hnc) in zip(groups, call_cols):
            vals[lc0 * 16:(lc0 + lnc) * 16] = SENT_LO
            vals[hc0 * 16:(hc0 + hnc) * 16] = SENT_HI
        m = (e_rank == r)
        mlo = m & is_lo
        mhi = m & ~is_lo
        p_lo = pos_base_lo[e_tile[mlo]] + slot[mlo] * 128 + e_part[mlo]
        vals[p_lo] = s_src_row[mlo]
        p_hi = pos_base_hi[e_tile[mhi]] + slot[mhi] * 128 + e_part[mhi]
        vals[p_hi] = s_src_row[mhi] - cfg.HI_BASE
        assert vals.min() >= 0 and vals.max() < 32768
        idx16[r] = np.tile(vals.reshape(-1, 16).T, (8, 1))

    return Sched(perm=perm, sortpos=sortpos, D_lo=D_lo, D_hi=D_hi,
                 groups=groups, idx16=idx16, call_cols=call_cols,
                 sub_off=sub_off, group_of=group_of)


def _bc(ap, shape):
    """broadcast an AP to shape (step-0 dims)"""
    return ap.broadcast_to(list(shape))


def build_program(cfg: Cfg, sch: Sched):
    """Build the single SPMD Bass program. Returns nc."""
    nc = bacc.Bacc("TRN2", target_bir_lowering=False, debug=False,
                   num_devices=cfg.R, num_swdge_queues=4)
    T, NPR, CHUNK, TROWS = cfg.T, cfg.NPR, cfg.CHUNK, cfg.TROWS
    HC1, H, HID, OUT = cfg.HC1, cfg.HEADS, cfg.HID, cfg.OUT
    ROW1, ROW2 = cfg.ROW1, cfg.ROW2
    TOTCOL = sch.idx16.shape[2]
    NT = T * 128

    # ---- I/O ----
    xT = nc.dram_tensor("xT", [cfg.F_IN, NT], F32, kind="ExternalInput")
    idxs_d = nc.dram_tensor("idxs", [128, TOTCOL], I16, kind="ExternalInput")
    W1_d = nc.dram_tensor("W1", [cfg.F_IN, HC1], F32, kind="ExternalInput")
    W1T_d = nc.dram_tensor("W1T", [HC1, cfg.F_IN], F32, kind="ExternalInput")
    A1s_d = nc.dram_tensor("A1s", [HC1, H], F32, kind="ExternalInput")
    A1d_d = nc.dram_tensor("A1d", [HC1, H], F32, kind="ExternalInput")
    B1_d = nc.dram_tensor("B1rep", [128, HC1], F32, kind="ExternalInput")
    W2_d = nc.dram_tensor("W2", [HC1, OUT], F32, kind="ExternalInput")
    W2T_d = nc.dram_tensor("W2T", [OUT, HC1], F32, kind="ExternalInput")
    a2s_d = nc.dram_tensor("a2s", [OUT, 1], F32, kind="ExternalInput")
    a2d_d = nc.dram_tensor("a2d", [OUT, 1], F32, kind="ExternalInput")
    B2_d = nc.dram_tensor("B2rep", [128, OUT], F32, kind="ExternalInput")
    out_d = nc.dram_tensor("out", [NT, OUT], F32, kind="ExternalOutput")

    KC = HC1 // 128   # contraction chunks over HC1 (2)

    with tile.TileContext(nc) as tc, ExitStack() as ctx:
        dram = ctx.enter_context(tc.tile_pool(name="dram", bufs=1, space="DRAM"))
        const = ctx.enter_context(tc.tile_pool(name="const", bufs=1))
        psum = ctx.enter_context(tc.tile_pool(name="psum", bufs=2, space="PSUM"))

        # DRAM scratch
        chunk1 = dram.tile([CHUNK, ROW1], BF16)
        table1 = dram.tile([TROWS, ROW1], BF16, addr_space="Shared")
        chunk2 = dram.tile([CHUNK, ROW2], BF16)
        table2 = dram.tile([TROWS, ROW2], BF16, addr_space="Shared")
        h1d = dram.tile([NT, HC1], BF16)

        # ---- persistent constants ----
        idx_s = const.tile([128, TOTCOL], I16, tag="idx")
        nc.sync.dma_start(idx_s[:, :], idxs_d[:, :])
        RHS1 = const.tile([128, HC1 + 2 * H], F32, tag="rhs1")
        nc.sync.dma_start(RHS1[:, 0:HC1], W1_d[:, :])
        B1_s = const.tile([128, HC1], F32, tag="b1")
        nc.sync.dma_start(B1_s[:, :], B1_d[:, :])
        B2_s = const.tile([128, OUT], F32, tag="b2")
        nc.sync.dma_start(B2_s[:, :], B2_d[:, :])
        RHS2 = const.tile([128, KC, OUT + 2], BF16, tag="rhs2")
        nc.gpsimd.dma_start(RHS2[:, :, 0:OUT],
                            W2_d.ap().rearrange("(k p) c -> p k c", p=128))
        arL = const.tile([128, T, H], F32, tag="arL")
        nc.vector.memset(arL[:, :, :], 0.0)
        ar2L = const.tile([128, T, 1], F32, tag="ar2L")
        nc.vector.memset(ar2L[:, :, :], 0.0)

        # ================= phase 1: projection + table 1 ====================
        TS_T = (NPR + 1 + 127) // 128
        full_t = NPR // 128
        rem = NPR - full_t * 128
        with tc.tile_pool(name="ph1", bufs=1) as ph1:
            xT_s = ph1.tile([128, NT], F32, tag="xT")
            nc.sync.dma_start(xT_s[:, :], xT[:, :])
            W1T_s = ph1.tile([128, KC, 128], F32, tag="w1t")
            nc.sync.dma_start(W1T_s[:, :, :],
                              W1T_d.ap().rearrange("(k p) f -> p k f", p=128))
            A1s_s = ph1.tile([128, KC, H], F32, tag="a1s")
            nc.sync.dma_start(A1s_s[:, :, :],
                              A1s_d.ap().rearrange("(k p) h -> p k h", p=128))
            A1d_s = ph1.tile([128, KC, H], F32, tag="a1d")
            nc.sync.dma_start(A1d_s[:, :, :],
                              A1d_d.ap().rearrange("(k p) h -> p k h", p=128))
            W2T_s = ph1.tile([OUT, HC1], F32, tag="w2t")
            nc.sync.dma_start(W2T_s[:, :], W2T_d[:, :])
            a2s_s = ph1.tile([OUT, 1], F32, tag="a2s")
            nc.sync.dma_start(a2s_s[:, :], a2s_d[:, :])
            a2d_s = ph1.tile([OUT, 1], F32, tag="a2d")
            nc.sync.dma_start(a2d_s[:, :], a2d_d[:, :])

            # fold attention vectors into projection RHS
            for (dst_off, A_s) in ((HC1, A1s_s), (HC1 + H, A1d_s)):
                ps = psum.tile([128, H], F32, tag="wprep")
                for k in range(KC):
                    nc.tensor.matmul(ps[:, :], W1T_s[:, k, :], A_s[:, k, :],
                                     start=(k == 0), stop=(k == KC - 1))
                nc.vector.tensor_copy(RHS1[:, dst_off:dst_off + H], ps[:, :])
            for (dst_off, a_s) in ((OUT, a2s_s), (OUT + 1, a2d_s)):
                for k in range(KC):
                    ps = psum.tile([128, 1], F32, tag="wprep2")
                    nc.tensor.matmul(ps[:, :], W2T_s[:, k * 128:(k + 1) * 128],
                                     a_s[:, :], start=True, stop=True)
                    nc.vector.tensor_copy(RHS2[:, k, dst_off:dst_off + 1],
                                          ps[:, :])

            tstage = ph1.tile([128, TS_T, ROW1], BF16, tag="tstage1")
            nc.vector.memset(tstage[:, :, :], 0.0)
            for t in range(T):
                ps = psum.tile([128, HC1 + 2 * H], F32, tag="proj1")
                nc.tensor.matmul(ps[:, :], xT_s[:, t * 128:(t + 1) * 128],
                                 RHS1[:, :], start=True, stop=True)
                nc.scalar.copy(tstage[:, t, 0:HC1], ps[:, 0:HC1])
                al_view = tstage[:, t, HC1:HC1 + 2 * H].bitcast(F32)
                nc.vector.tensor_copy(al_view[:, :], ps[:, HC1:HC1 + H])
                nc.vector.tensor_copy(arL[:, t, :],
                                      ps[:, HC1 + H:HC1 + 2 * H])
            # sentinel row -> chunk row 0 (h = 0, al = -1e30)
            sent1 = ph1.tile([1, ROW1], BF16, tag="sent1")
            nc.vector.memset(sent1[:, :], 0.0)
            nc.vector.memset(sent1[:, HC1:HC1 + 2 * H].bitcast(F32), AL_SENT)
            nc.sync.dma_start(chunk1[0:1, :], sent1[:, :])
            nc.sync.dma_start(
                chunk1[1:1 + full_t * 128, 0:HC1 + 2 * H].rearrange(
                    "(t p) c -> p t c", p=128),
                tstage[:, 0:full_t, 0:HC1 + 2 * H])
            if rem > 0:
                nc.sync.dma_start(chunk1[1 + full_t * 128:CHUNK, 0:HC1 + 2 * H],
                                  tstage[0:rem, full_t, 0:HC1 + 2 * H])
        nc.gpsimd.collective_compute(
            "AllGather", Alu.bypass,
            replica_groups=[list(range(cfg.R))],
            ins=[chunk1[:, :].opt()], outs=[table1[:, :].opt()])

        epool = ctx.enter_context(tc.tile_pool(name="edge", bufs=1))
        gpool = ctx.enter_context(tc.tile_pool(name="gpool", bufs=3))
        spool = ctx.enter_context(tc.tile_pool(name="spool", bufs=3))
        ppool = ctx.enter_context(tc.tile_pool(name="ppool", bufs=2))

        # ================= edge phase (shared for both layers) ==============
        def edge_layer(layer, table, ROW, CH, NH, arl_ap, out_cb):
            """layer: 1 or 2. CH: channels per head (32 / 64). NH: heads.
            arl_ap(t) -> [128, NH] f32 AP; out_cb(t, unn, rec) emits epilogue.
            """
            HCL = CH * NH
            lo_tab = table[0:cfg.LO_LIM, :]
            hi_tab = table[cfg.HI_BASE:TROWS, :]
            for gi, ((t0, t1), (lc0, lnc, hc0, hnc)) in enumerate(
                    zip(sch.groups, sch.call_cols)):
                S_lo = int(sch.D_lo[t0:t1].sum())
                S_hi = int(sch.D_hi[t0:t1].sum())
                S = S_lo + S_hi
                g = gpool.tile([128, cfg.SLOT_CAP, ROW], BF16, tag="gbuf")
                nc.gpsimd.dma_gather(
                    g[:, 0:S_lo, :], lo_tab, idx_s[:, lc0:lc0 + lnc],
                    num_idxs=S_lo * 128, num_idxs_reg=S_lo * 128,
                    elem_size=ROW, elem_step=ROW, single_packet=False,
                    queue_num=(2 * gi) % 4)
                nc.gpsimd.dma_gather(
                    g[:, S_lo:S, :], hi_tab, idx_s[:, hc0:hc0 + hnc],
                    num_idxs=S_hi * 128, num_idxs_reg=S_hi * 128,
                    elem_size=ROW, elem_step=ROW, single_packet=False,
                    queue_num=(2 * gi + 1) % 4)

                for t in range(t0, t1):
                    parts, dens = [], []
                    for reg in (0, 1):
                        D = int((sch.D_lo, sch.D_hi)[reg][t])
                        so = int(sch.sub_off[t, reg])
                        gs = g[:, so:so + D, :]
                        # e = leakyrelu(al + ar)
                        e = spool.tile([128, cfg.SLOT_CAP, NH], F32, tag="e")
                        al = gs[:, :, HCL:HCL + 2 * NH].bitcast(F32)
                        nc.vector.tensor_add(
                            e[:, 0:D, :], al,
                            _bc(arl_ap(t).unsqueeze(1), (128, D, NH)))
                        nc.vector.scalar_tensor_tensor(
                            e[:, 0:D, :], e[:, 0:D, :], cfg.NEG, e[:, 0:D, :],
                            op0=Alu.mult, op1=Alu.max)
                        # p = exp(e)  (bf16 out)
                        p = spool.tile([128, cfg.SLOT_CAP, NH], BF16, tag="p")
                        nc.scalar.activation(p[:, 0:D, :], e[:, 0:D, :],
                                             Act.Exp)
                        # denom partial = sum over slots
                        den = spool.tile([128, NH], F32, tag="den")
                        nc.vector.tensor_reduce(
                            den[:, :], p[:, 0:D, :].transpose([0, 2, 1]),
                            axis=mybir.AxisListType.X, op=Alu.add)
                        dens.append(den)
                        # msg = h * p (p broadcast over channels, step-0 AP)
                        msg = ppool.tile([128, cfg.SLOT_CAP, NH, CH], BF16,
                                         tag="msg")
                        pb = p[:, 0:D, :].unsqueeze(3).broadcast_to(
                            [128, D, NH, CH])
                        nc.vector.tensor_mul(
                            msg[:, 0:D, :, :],
                            gs[:, :, 0:HCL].rearrange("p d (h c) -> p d h c",
                                                      h=NH),
                            pb)
                        msg = msg[:, :, :, :].rearrange("p d h c -> p d (h c)")
                        # tree-sum over slots -> part [128, HCL] f32
                        part = spool.tile([128, HCL], F32, tag="part")
                        cur = D
                        while cur > 2:
                            hh = cur // 2
                            nc.vector.tensor_add(
                                msg[:, 0:hh, :], msg[:, 0:hh, :],
                                msg[:, cur - hh:cur, :])
                            cur -= hh
                        if cur == 2:
                            nc.vector.tensor_add(part[:, :], msg[:, 0, :],
                                                 msg[:, 1, :])
                        else:
                            nc.vector.tensor_copy(part[:, :], msg[:, 0, :])
                        parts.append(part)
                    unn = spool.tile([128, HCL], F32, tag="unn")
                    nc.vector.tensor_add(unn[:, :], parts[0][:, :],
                                         parts[1][:, :])
                    den = spool.tile([128, NH], F32, tag="dent")
                    nc.vector.tensor_add(den[:, :], dens[0][:, :],
                                         dens[1][:, :])
                    # den >= exp(leakyrelu(self-loop logit)) > 0: every dst
                    # has a self-loop and |e| is O(1), so no eps guard needed
                    rec = spool.tile([128, NH], F32, tag="rec")
                    nc.vector.reciprocal(rec[:, :], den[:, :])
                    out_cb(t, unn, rec)

        # ---- L1 epilogue: normalize, +b1, ELU, store h1 ----
        def l1_out(t, unn, rec):
            y = spool.tile([128, HC1], F32, tag="y1")
            nc.vector.tensor_mul(
                y.rearrange("p (h c) -> p h c", h=H),
                unn.rearrange("p (h c) -> p h c", h=H),
                _bc(rec[:, :].unsqueeze(2), (128, H, HID)))
            nc.vector.tensor_add(y[:, :], y[:, :], B1_s[:, :])
            mn = spool.tile([128, HC1], F32, tag="mn1")
            nc.vector.tensor_scalar_min(mn[:, :], y[:, :], 0.0)
            nc.vector.tensor_scalar_max(y[:, :], y[:, :], 0.0)
            em = spool.tile([128, HC1], F32, tag="em1")
            nc.scalar.activation(em[:, :], mn[:, :], Act.Exp)
            h1t = spool.tile([128, HC1], BF16, tag="h1t")
            nc.vector.scalar_tensor_tensor(h1t[:, :], em[:, :], -1.0, y[:, :],
                                           op0=Alu.add, op1=Alu.add)
            nc.sync.dma_start(h1d[t * 128:(t + 1) * 128, :], h1t[:, :])

        edge_layer(1, table1, ROW1, HID, H, lambda t: arL[:, t, :], l1_out)

        # ---- L2 projection from h1 (DMA-transpose h1d) ----
        h1T = epool.tile([128, KC, NT], BF16, tag="h1T")
        for k in range(KC):
            nc.sync.dma_start_transpose(h1T[:, k, :],
                                        h1d[:, k * 128:(k + 1) * 128])
        tstage2 = epool.tile([128, TS_T, ROW2], BF16, tag="tstage2")
        nc.vector.memset(tstage2[:, :, :], 0.0)
        for t in range(T):
            ps = psum.tile([128, OUT + 2], F32, tag="proj2")
            for k in range(KC):
                nc.tensor.matmul(ps[:, :], h1T[:, k, t * 128:(t + 1) * 128],
                                 RHS2[:, k, :], start=(k == 0),
                                 stop=(k == KC - 1))
            nc.scalar.copy(tstage2[:, t, 0:OUT], ps[:, 0:OUT])
            al2_view = tstage2[:, t, OUT:OUT + 2].bitcast(F32)
            nc.vector.tensor_copy(al2_view[:, :], ps[:, OUT:OUT + 1])
            nc.vector.tensor_copy(ar2L[:, t, :], ps[:, OUT + 1:OUT + 2])
        sent2 = epool.tile([1, ROW2], BF16, tag="sent2")
        nc.vector.memset(sent2[:, :], 0.0)
        nc.vector.memset(sent2[:, OUT:OUT + 2].bitcast(F32), AL_SENT)
        nc.sync.dma_start(chunk2[0:1, :], sent2[:, :])
        nc.sync.dma_start(
            chunk2[1:1 + full_t * 128, 0:OUT + 2].rearrange(
                "(t p) c -> p t c", p=128),
            tstage2[:, 0:full_t, 0:OUT + 2])
        if rem > 0:
            nc.sync.dma_start(chunk2[1 + full_t * 128:CHUNK, 0:OUT + 2],
                              tstage2[0:rem, full_t, 0:OUT + 2])
        nc.gpsimd.collective_compute(
            "AllGather", Alu.bypass,
            replica_groups=[list(range(cfg.R))],
            ins=[chunk2[:, :].opt()], outs=[table2[:, :].opt()])

        # ---- L2 epilogue: normalize, +b2, log_softmax, store out ----
        ostage = epool.tile([128, T, OUT], F32, tag="ostage")

        def l2_out(t, unn, rec):
            y = spool.tile([128, OUT], F32, tag="y2")
            nc.vector.tensor_scalar_mul(y[:, :], unn[:, :], rec[:, 0:1])
            nc.vector.tensor_add(y[:, :], y[:, :], B2_s[:, :])
            # |y| is O(10): exp stays finite in f32, no max-subtract needed
            ex = spool.tile([128, OUT], F32, tag="ex2")
            ssum = spool.tile([128, 1], F32, tag="ss2")
            nc.scalar.activation(ex[:, :], y[:, :], Act.Exp,
                                 accum_out=ssum[:, :])
            ls = spool.tile([128, 1], F32, tag="ls2")
            nc.scalar.activation(ls[:, :], ssum[:, :], Act.Ln)
            nc.vector.tensor_scalar_sub(ostage[:, t, :], y[:, :], ls[:, 0:1])

        edge_layer(2, table2, ROW2, OUT, 1, lambda t: ar2L[:, t, :], l2_out)
        nc.sync.dma_start(out_d.ap().rearrange("(t p) c -> p t c", p=128),
                          ostage[:, :, :])

    nc.compile()
    return nc


def _host_inputs(cfg: Cfg, sch: Sched, inputs: dict):
    """Build per-rank in_maps from the full problem inputs."""
    x = np.asarray(inputs["x"], np.float32)
    W1 = np.asarray(inputs["W1"], np.float32)
    a1_src = np.asarray(inputs["a1_src"], np.float32)
    a1_dst = np.asarray(inputs["a1_dst"], np.float32)
    b1 = np.asarray(inputs["b1"], np.float32)
    W2 = np.asarray(inputs["W2"], np.float32)
    a2_src = np.asarray(inputs["a2_src"], np.float32)
    a2_dst = np.asarray(inputs["a2_dst"], np.float32)
    b2 = np.asarray(inputs["b2"], np.float32)
    H, HID, HC1, OUT = cfg.HEADS, cfg.HID, cfg.HC1, cfg.OUT

    # block-diagonal per-head attention matrices: al = h @ A1s
    A1s = np.zeros((HC1, H), np.float32)
    A1d = np.zeros((HC1, H), np.float32)
    for h in range(H):
        A1s[h * HID:(h + 1) * HID, h] = a1_src[h]
        A1d[h * HID:(h + 1) * HID, h] = a1_dst[h]

    common = {
        "W1": np.ascontiguousarray(W1),
        "W1T": np.ascontiguousarray(W1.T),
        "A1s": A1s, "A1d": A1d,
        "B1rep": np.tile(b1[None, :], (128, 1)).astype(np.float32),
        "W2": np.ascontiguousarray(W2),
        "W2T": np.ascontiguousarray(W2.T),
        "a2s": np.ascontiguousarray(a2_src.reshape(OUT, 1)),
        "a2d": np.ascontiguousarray(a2_dst.reshape(OUT, 1)),
        "B2rep": np.tile(b2[None, :], (128, 1)).astype(np.float32),
    }
    in_maps = []
    for r in range(cfg.R):
        m = dict(common)
        xp = np.zeros((cfg.T * 128, x.shape[1]), np.float32)
        xp[:cfg.NPR] = x[sch.perm[r]]
        m["xT"] = np.ascontiguousarray(xp.T)
        m["idxs"] = np.ascontiguousarray(sch.idx16[r])
        in_maps.append(m)
    return in_maps


def run(cfg: Cfg, inputs: dict, trace: bool = False):
    edge_index = np.asarray(inputs["edge_index"])
    loops = np.arange(cfg.N, dtype=edge_index.dtype)
    src = np.concatenate([edge_index[0], loops]).astype(np.int64)
    dst = np.concatenate([edge_index[1], loops]).astype(np.int64)

    sch = build_schedule(cfg, src, dst)
    nc = build_program(cfg, sch)
    in_maps = _host_inputs(cfg, sch, inputs)
    res = bass_utils.run_bass_kernel_spmd(
        nc, in_maps, core_ids=list(range(cfg.R)), trace=trace)
    out = np.empty((cfg.N, cfg.OUT), np.float32)
    for r in range(cfg.R):
        o = res.results[r]["out"]
        out[sch.perm[r]] = o[:cfg.NPR]
    return out, res


def kernel(**inputs) -> np.ndarray:
    cfg = Cfg()
    out, _ = run(cfg, inputs)
    return out


if __name__ == "__main__":
    import reference
    inputs = {k: np.asarray(v) for k, v in reference.setup_inputs().items()}
    out = kernel(**inputs)
    exp = np.asarray(reference.reference(**reference.setup_inputs()))
    err = np.abs(out - exp).max() / (np.abs(exp).max() + 1e-12)
    print("rel err:", err)


# revision 33
# speedup vs baseline: 1.0185x; 1.0016x over previous
"""2-layer GAT (gnn_message_passing) on 8 TRN2 NeuronCores.

Strategy (graph/data parallel, per sharding hint):
  - Nodes are partitioned across 8 ranks (6250 dst nodes each). Each rank owns
    the segment-softmax + aggregation for its destination nodes.
  - Per layer, every rank computes the projected features (h = x @ W,
    attention source/dest logits al/ar fused into the same matmul via an
    augmented RHS) for ITS OWN nodes, writes them as rows of a gather table
    (768B rows for layer 1: 256 bf16 h + 8 f32 al; 256B rows for layer 2),
    then an AllGather replicates the full table to every rank.
  - Edge stage: destinations are degree-sorted and packed into tiles of 128
    (dst on partitions); each dst gets a padded run of incoming-edge "slots"
    along the free dimension. Source rows are fetched with dma_gather
    (SWDGE indexed gather). Since gather indices are int16, the table is
    addressed through two base windows (rows [0,32768) and
    [TROWS-32768, TROWS)), and each dst's slots are split into a "lo" and
    "hi" range accordingly. Tiles are greedily grouped (up to SLOT_CAP
    slots) into shared gather calls, and the lo/hi gathers of consecutive
    groups rotate over 4 SWDGE queues so their drains overlap.
  - Slot-grid math per (tile, region): e = leakyrelu(al_src + ar_dst),
    p = exp(e) (no max-subtract needed at these magnitudes), denom =
    free-dim reduce, normalization applied AFTER aggregation.
  - Aggregation: msg = h_src * p (p broadcast over channels via a step-0
    AP), then a pairwise tree of wide tensor adds along the slot dim.
  - Padding slots read a sentinel table row (h = 0, al = -1e30 -> p = 0).

The full output is assembled on the host from the 8 per-rank outputs
(undoing the degree-sort permutation).
"""

import sys
from contextlib import ExitStack
from dataclasses import dataclass

import numpy as np

for _p in ("/opt/trn_rl_repo",):
    if _p not in sys.path:
        sys.path.insert(0, _p)

import concourse.bass as bass
import concourse.bacc as bacc
import concourse.mybir as mybir
import concourse.tile as tile
from concourse import bass_utils

F32 = mybir.dt.float32
BF16 = mybir.dt.bfloat16
I16 = mybir.dt.int16
AL_SENT = -1.0e30
Alu = mybir.AluOpType
Act = mybir.ActivationFunctionType


@dataclass
class Cfg:
    N: int = 50000
    E: int = 500000          # edges before self-loops
    F_IN: int = 128
    HID: int = 32
    HEADS: int = 8
    OUT: int = 64
    NEG: float = 0.2
    R: int = 8
    SLOT_CAP: int = 32       # max slots per gather group (SBUF budget)
    hi_base: int = -1        # -1: auto (TROWS - 32768, clamped to >= 0)

    @property
    def HC1(self):
        return self.HEADS * self.HID     # 256

    @property
    def NPR(self):
        return self.N // self.R

    @property
    def CHUNK(self):
        return self.NPR + 1              # + sentinel row

    @property
    def TROWS(self):
        return self.R * self.CHUNK

    @property
    def T(self):
        return (self.NPR + 127) // 128   # dst tiles per rank

    @property
    def ROW1(self):
        return 384                       # bf16 elems: 256 h + 16 (8xf32 al) + pad

    @property
    def ROW2(self):
        return 128                       # bf16 elems: 64 h2 + 2 (1xf32 al2) + pad

    @property
    def HI_BASE(self):
        if self.hi_base >= 0:
            return self.hi_base
        return max(0, self.TROWS - 32768)

    @property
    def LO_LIM(self):
        # rows addressable from base 0
        return min(self.TROWS, 32768)


@dataclass
class Sched:
    perm: np.ndarray          # [R, NPR] perm[r][pos] = global node id
    sortpos: np.ndarray       # [N] position of node within its rank
    D_lo: np.ndarray          # [T]
    D_hi: np.ndarray          # [T]
    groups: list              # list of (t0, t1) tile ranges
    idx16: np.ndarray         # [R, 128, TOTCOL] int16
    call_cols: list           # per group: (lo_col0, lo_ncol, hi_col0, hi_ncol)
    sub_off: np.ndarray       # [T, 2] slot offset of (tile, region) in group buffer
    group_of: np.ndarray      # [T] group index of tile


def build_schedule(cfg: Cfg, src: np.ndarray, dst: np.ndarray) -> Sched:
    N, R, NPR, CHUNK, T = cfg.N, cfg.R, cfg.NPR, cfg.CHUNK, cfg.T
    deg = np.bincount(dst, minlength=N).astype(np.int64)
    odeg = np.bincount(src, minlength=N).astype(np.int64)

    # assign the highest out-degree nodes to the ranks whose table chunks sit
    # in the lo/hi window overlap, maximizing the flexible-edge fraction
    oorder = np.argsort(-odeg, kind="stable")
    fill_order = [3, 4, 2, 5, 1, 6, 0, 7]
    rank_of = np.empty(N, np.int64)
    for i, r in enumerate(fill_order):
        rank_of[oorder[i * NPR:(i + 1) * NPR]] = r

    sortpos = np.empty(N, np.int64)
    perm = np.empty((R, NPR), np.int64)
    for r in range(R):
        nodes = np.where(rank_of == r)[0]
        order = np.argsort(-deg[nodes], kind="stable")
        perm[r] = nodes[order]
        sortpos[perm[r]] = np.arange(NPR)
    # chunk row 0 of every rank is its sentinel row; real rows start at 1
    row_of = rank_of * CHUNK + 1 + sortpos        # [N] table row of each node

    src_row = row_of[src]
    # categories: 0 = forced lo, 1 = flexible, 2 = forced hi
    cat = np.where(src_row < cfg.HI_BASE, 0, np.where(src_row < cfg.LO_LIM, 1, 2))

    # global dst key: (rank, sorted position)
    dkey = rank_of[dst] * NPR + sortpos[dst]
    order = np.lexsort((cat, dkey))
    s_src_row = src_row[order]
    s_dkey = dkey[order]

    cnt = np.bincount(dkey, minlength=R * NPR)
    cnt_lo = np.bincount(dkey[cat == 0], minlength=R * NPR)
    cnt_hi = np.bincount(dkey[cat == 2], minlength=R * NPR)
    start = np.concatenate([[0], np.cumsum(cnt)])[:-1]

    # per-dst lo count: balance towards half, respecting forced counts
    half = (cnt + 1) // 2
    nlo = np.clip(half, cnt_lo, cnt - cnt_hi)
    nhi = cnt - nlo

    pos_in_dst = np.arange(len(order)) - start[s_dkey]
    is_lo = pos_in_dst < nlo[s_dkey]
    slot = np.where(is_lo, pos_in_dst, pos_in_dst - nlo[s_dkey])

    # slot grid shared by all ranks
    D_lo = np.zeros(T, np.int64)
    D_hi = np.zeros(T, np.int64)
    nlo_g = nlo.reshape(R, NPR)
    nhi_g = nhi.reshape(R, NPR)
    for t in range(T):
        sl = slice(t * 128, min((t + 1) * 128, NPR))
        D_lo[t] = max(1, nlo_g[:, sl].max())
        D_hi[t] = max(1, nhi_g[:, sl].max())
    assert (D_lo + D_hi).max() <= cfg.SLOT_CAP, (
        f"tile needs {(D_lo + D_hi).max()} slots > SLOT_CAP {cfg.SLOT_CAP}")

    # greedy grouping of tiles, capped at SLOT_CAP slots
    groups = []
    group_of = np.zeros(T, np.int64)
    t0 = 0
    while t0 < T:
        t1 = t0 + 1
        tot = D_lo[t0] + D_hi[t0]
        while t1 < T and tot + D_lo[t1] + D_hi[t1] <= cfg.SLOT_CAP:
            tot += D_lo[t1] + D_hi[t1]
            t1 += 1
        group_of[t0:t1] = len(groups)
        groups.append((t0, t1))
        t0 = t1

    # slot offsets of each (tile, region) within its group buffer:
    # [lo slots of t0 | lo t1 | ... | hi t0 | hi t1 | ...]
    sub_off = np.zeros((T, 2), np.int64)
    call_cols = []
    col = 0
    pos_base_lo = np.zeros(T, np.int64)
    pos_base_hi = np.zeros(T, np.int64)
    for (t0, t1) in groups:
        S_lo = int(D_lo[t0:t1].sum())
        S_hi = int(D_hi[t0:t1].sum())
        off = 0
        lo_col0 = col
        for t in range(t0, t1):
            sub_off[t, 0] = off
            pos_base_lo[t] = col * 16 + off * 128
            off += D_lo[t]
        col += S_lo * 8  # 128/16 columns per slot-column
        hi_col0 = col
        off2 = 0
        for t in range(t0, t1):
            sub_off[t, 1] = S_lo + off2
            pos_base_hi[t] = col * 16 + off2 * 128
            off2 += D_hi[t]
        col += S_hi * 8
        call_cols.append((lo_col0, S_lo * 8, hi_col0, S_hi * 8))
    TOTCOL = col
    TOTPOS = TOTCOL * 16

    SENT_LO = 0                                    # rank 0 sentinel row
    SENT_HI = (R - 1) * CHUNK - cfg.HI_BASE        # last rank sentinel, local
    assert 0 <= SENT_HI < 32768

    # fill idx values per rank
    e_rank = s_dkey // NPR
    e_pos = s_dkey % NPR
    e_tile = e_pos // 128
    e_part = e_pos % 128
    idx16 = np.empty((R, 128, TOTCOL), np.int16)
    for r in range(R):
        vals = np.empty(TOTPOS, np.int32)
        for (t0, t1), (lc0, lnc, hc0, hnc) in zip(groups, call_cols):
            vals[lc0 * 16:(lc0 + lnc) * 16] = SENT_LO
            vals[hc0 * 16:(hc0 + hnc) * 16] = SENT_HI
        m = (e_rank == r)
        mlo = m & is_lo
        mhi = m & ~is_lo
        p_lo = pos_base_lo[e_tile[mlo]] + slot[mlo] * 128 + e_part[mlo]
        vals[p_lo] = s_src_row[mlo]
        p_hi = pos_base_hi[e_tile[mhi]] + slot[mhi] * 128 + e_part[mhi]
        vals[p_hi] = s_src_row[mhi] - cfg.HI_BASE
        assert vals.min() >= 0 and vals.max() < 32768
        idx16[r] = np.tile(vals.reshape(-1, 16).T, (8, 1))

    return Sched(perm=perm, sortpos=sortpos, D_lo=D_lo, D_hi=D_hi,
                 groups=groups, idx16=idx16, call_cols=call_cols,
                 sub_off=sub_off, group_of=group_of)


def _bc(ap, shape):
    """broadcast an AP to shape (step-0 dims)"""
    return ap.broadcast_to(list(shape))


def build_program(cfg: Cfg, sch: Sched):
    """Build the single SPMD Bass program. Returns nc."""
    nc = bacc.Bacc("TRN2", target_bir_lowering=False, debug=False,
                   num_devices=cfg.R, num_swdge_queues=4)
    T, NPR, CHUNK, TROWS = cfg.T, cfg.NPR, cfg.CHUNK, cfg.TROWS
    HC1, H, HID, OUT = cfg.HC1, cfg.HEADS, cfg.HID, cfg.OUT
    ROW1, ROW2 = cfg.ROW1, cfg.ROW2
    TOTCOL = sch.idx16.shape[2]
    NT = T * 128

    # ---- I/O ----
    xT = nc.dram_tensor("xT", [cfg.F_IN, NT], F32, kind="ExternalInput")
    idxs_d = nc.dram_tensor("idxs", [128, TOTCOL], I16, kind="ExternalInput")
    W1_d = nc.dram_tensor("W1", [cfg.F_IN, HC1], F32, kind="ExternalInput")
    W1T_d = nc.dram_tensor("W1T", [HC1, cfg.F_IN], F32, kind="ExternalInput")
    A1s_d = nc.dram_tensor("A1s", [HC1, H], F32, kind="ExternalInput")
    A1d_d = nc.dram_tensor("A1d", [HC1, H], F32, kind="ExternalInput")
    B1_d = nc.dram_tensor("B1rep", [128, HC1], F32, kind="ExternalInput")
    W2_d = nc.dram_tensor("W2", [HC1, OUT], F32, kind="ExternalInput")
    W2T_d = nc.dram_tensor("W2T", [OUT, HC1], F32, kind="ExternalInput")
    a2s_d = nc.dram_tensor("a2s", [OUT, 1], F32, kind="ExternalInput")
    a2d_d = nc.dram_tensor("a2d", [OUT, 1], F32, kind="ExternalInput")
    B2_d = nc.dram_tensor("B2rep", [128, OUT], F32, kind="ExternalInput")
    out_d = nc.dram_tensor("out", [NT, OUT], F32, kind="ExternalOutput")

    KC = HC1 // 128   # contraction chunks over HC1 (2)

    with tile.TileContext(nc) as tc, ExitStack() as ctx:
        dram = ctx.enter_context(tc.tile_pool(name="dram", bufs=1, space="DRAM"))
        const = ctx.enter_context(tc.tile_pool(name="const", bufs=1))
        psum = ctx.enter_context(tc.tile_pool(name="psum", bufs=2, space="PSUM"))

        # DRAM scratch
        chunk1 = dram.tile([CHUNK, ROW1], BF16)
        table1 = dram.tile([TROWS, ROW1], BF16, addr_space="Shared")
        chunk2 = dram.tile([CHUNK, ROW2], BF16)
        table2 = dram.tile([TROWS, ROW2], BF16, addr_space="Shared")
        h1d = dram.tile([NT, HC1], BF16)

        # ---- persistent constants ----
        idx_s = const.tile([128, TOTCOL], I16, tag="idx")
        nc.sync.dma_start(idx_s[:, :], idxs_d[:, :])
        RHS1 = const.tile([128, HC1 + 2 * H], F32, tag="rhs1")
        nc.sync.dma_start(RHS1[:, 0:HC1], W1_d[:, :])
        B1_s = const.tile([128, HC1], F32, tag="b1")
        nc.sync.dma_start(B1_s[:, :], B1_d[:, :])
        B2_s = const.tile([128, OUT], F32, tag="b2")
        nc.sync.dma_start(B2_s[:, :], B2_d[:, :])
        RHS2 = const.tile([128, KC, OUT + 2], BF16, tag="rhs2")
        nc.gpsimd.dma_start(RHS2[:, :, 0:OUT],
                            W2_d.ap().rearrange("(k p) c -> p k c", p=128))
        arL = const.tile([128, T, H], F32, tag="arL")
        nc.vector.memset(arL[:, :, :], 0.0)
        ar2L = const.tile([128, T, 1], F32, tag="ar2L")
        nc.vector.memset(ar2L[:, :, :], 0.0)

        # ================= phase 1: projection + table 1 ====================
        TS_T = (NPR + 1 + 127) // 128
        full_t = NPR // 128
        rem = NPR - full_t * 128
        with tc.tile_pool(name="ph1", bufs=1) as ph1:
            xT_s = ph1.tile([128, NT], F32, tag="xT")
            nc.sync.dma_start(xT_s[:, :], xT[:, :])
            W1T_s = ph1.tile([128, KC, 128], F32, tag="w1t")
            nc.sync.dma_start(W1T_s[:, :, :],
                              W1T_d.ap().rearrange("(k p) f -> p k f", p=128))
            A1s_s = ph1.tile([128, KC, H], F32, tag="a1s")
            nc.sync.dma_start(A1s_s[:, :, :],
                              A1s_d.ap().rearrange("(k p) h -> p k h", p=128))
            A1d_s = ph1.tile([128, KC, H], F32, tag="a1d")
            nc.sync.dma_start(A1d_s[:, :, :],
                              A1d_d.ap().rearrange("(k p) h -> p k h", p=128))
            W2T_s = ph1.tile([OUT, HC1], F32, tag="w2t")
            nc.sync.dma_start(W2T_s[:, :], W2T_d[:, :])
            a2s_s = ph1.tile([OUT, 1], F32, tag="a2s")
            nc.sync.dma_start(a2s_s[:, :], a2s_d[:, :])
            a2d_s = ph1.tile([OUT, 1], F32, tag="a2d")
            nc.sync.dma_start(a2d_s[:, :], a2d_d[:, :])

            # fold attention vectors into projection RHS
            for (dst_off, A_s) in ((HC1, A1s_s), (HC1 + H, A1d_s)):
                ps = psum.tile([128, H], F32, tag="wprep")
                for k in range(KC):
                    nc.tensor.matmul(ps[:, :], W1T_s[:, k, :], A_s[:, k, :],
                                     start=(k == 0), stop=(k == KC - 1))
                nc.vector.tensor_copy(RHS1[:, dst_off:dst_off + H], ps[:, :])
            for (dst_off, a_s) in ((OUT, a2s_s), (OUT + 1, a2d_s)):
                for k in range(KC):
                    ps = psum.tile([128, 1], F32, tag="wprep2")
                    nc.tensor.matmul(ps[:, :], W2T_s[:, k * 128:(k + 1) * 128],
                                     a_s[:, :], start=True, stop=True)
                    nc.vector.tensor_copy(RHS2[:, k, dst_off:dst_off + 1],
                                          ps[:, :])

            tstage = ph1.tile([128, TS_T, ROW1], BF16, tag="tstage1")
            nc.vector.memset(tstage[:, :, :], 0.0)
            for t in range(T):
                ps = psum.tile([128, HC1 + 2 * H], F32, tag="proj1")
                nc.tensor.matmul(ps[:, :], xT_s[:, t * 128:(t + 1) * 128],
                                 RHS1[:, :], start=True, stop=True)
                nc.scalar.copy(tstage[:, t, 0:HC1], ps[:, 0:HC1])
                al_view = tstage[:, t, HC1:HC1 + 2 * H].bitcast(F32)
                nc.vector.tensor_copy(al_view[:, :], ps[:, HC1:HC1 + H])
                nc.vector.tensor_copy(arL[:, t, :],
                                      ps[:, HC1 + H:HC1 + 2 * H])
            # sentinel row -> chunk row 0 (h = 0, al = -1e30)
            sent1 = ph1.tile([1, ROW1], BF16, tag="sent1")
            nc.vector.memset(sent1[:, :], 0.0)
            nc.vector.memset(sent1[:, HC1:HC1 + 2 * H].bitcast(F32), AL_SENT)
            nc.sync.dma_start(chunk1[0:1, :], sent1[:, :])
            nc.sync.dma_start(
                chunk1[1:1 + full_t * 128, 0:HC1 + 2 * H].rearrange(
                    "(t p) c -> p t c", p=128),
                tstage[:, 0:full_t, 0:HC1 + 2 * H])
            if rem > 0:
                nc.sync.dma_start(chunk1[1 + full_t * 128:CHUNK, 0:HC1 + 2 * H],
                                  tstage[0:rem, full_t, 0:HC1 + 2 * H])
        nc.gpsimd.collective_compute(
            "AllGather", Alu.bypass,
            replica_groups=[list(range(cfg.R))],
            ins=[chunk1[:, :].opt()], outs=[table1[:, :].opt()])

        epool = ctx.enter_context(tc.tile_pool(name="edge", bufs=1))
        gpool = ctx.enter_context(tc.tile_pool(name="gpool", bufs=3))
        spool = ctx.enter_context(tc.tile_pool(name="spool", bufs=3))
        ppool = ctx.enter_context(tc.tile_pool(name="ppool", bufs=2))

        # ================= edge phase (shared for both layers) ==============
        def edge_layer(layer, table, ROW, CH, NH, arl_ap, out_cb):
            """layer: 1 or 2. CH: channels per head (32 / 64). NH: heads.
            arl_ap(t) -> [128, NH] f32 AP; out_cb(t, unn, rec) emits epilogue.
            """
            HCL = CH * NH
            lo_tab = table[0:cfg.LO_LIM, :]
            hi_tab = table[cfg.HI_BASE:TROWS, :]
            for gi, ((t0, t1), (lc0, lnc, hc0, hnc)) in enumerate(
                    zip(sch.groups, sch.call_cols)):
                S_lo = int(sch.D_lo[t0:t1].sum())
                S_hi = int(sch.D_hi[t0:t1].sum())
                S = S_lo + S_hi
                g = gpool.tile([128, cfg.SLOT_CAP, ROW], BF16, tag="gbuf")
                nc.gpsimd.dma_gather(
                    g[:, 0:S_lo, :], lo_tab, idx_s[:, lc0:lc0 + lnc],
                    num_idxs=S_lo * 128, num_idxs_reg=S_lo * 128,
                    elem_size=ROW, elem_step=ROW, single_packet=False,
                    queue_num=(2 * gi) % 4)
                nc.gpsimd.dma_gather(
                    g[:, S_lo:S, :], hi_tab, idx_s[:, hc0:hc0 + hnc],
                    num_idxs=S_hi * 128, num_idxs_reg=S_hi * 128,
                    elem_size=ROW, elem_step=ROW, single_packet=False,
                    queue_num=(2 * gi + 1) % 4)

                for t in range(t0, t1):
                    parts, dens = [], []
                    for reg in (0, 1):
                        D = int((sch.D_lo, sch.D_hi)[reg][t])
                        so = int(sch.sub_off[t, reg])
                        gs = g[:, so:so + D, :]
                        # e = leakyrelu(al + ar)
                        e = spool.tile([128, cfg.SLOT_CAP, NH], F32, tag="e")
                        al = gs[:, :, HCL:HCL + 2 * NH].bitcast(F32)
                        nc.vector.tensor_add(
                            e[:, 0:D, :], al,
                            _bc(arl_ap(t).unsqueeze(1), (128, D, NH)))
                        nc.vector.scalar_tensor_tensor(
                            e[:, 0:D, :], e[:, 0:D, :], cfg.NEG, e[:, 0:D, :],
                            op0=Alu.mult, op1=Alu.max)
                        # p = exp(e)  (bf16 out)
                        p = spool.tile([128, cfg.SLOT_CAP, NH], BF16, tag="p")
                        nc.scalar.activation(p[:, 0:D, :], e[:, 0:D, :],
                                             Act.Exp)
                        # denom partial = sum over slots
                        den = spool.tile([128, NH], F32, tag="den")
                        nc.vector.tensor_reduce(
                            den[:, :], p[:, 0:D, :].transpose([0, 2, 1]),
                            axis=mybir.AxisListType.X, op=Alu.add)
                        dens.append(den)
                        # msg = h * p (p broadcast over channels, step-0 AP)
                        msg = ppool.tile([128, cfg.SLOT_CAP, NH, CH], BF16,
                                         tag="msg")
                        pb = p[:, 0:D, :].unsqueeze(3).broadcast_to(
                            [128, D, NH, CH])
                        nc.vector.tensor_mul(
                            msg[:, 0:D, :, :],
                            gs[:, :, 0:HCL].rearrange("p d (h c) -> p d h c",
                                                      h=NH),
                            pb)
                        msg = msg[:, :, :, :].rearrange("p d h c -> p d (h c)")
                        # tree-sum over slots -> part [128, HCL] f32
                        part = spool.tile([128, HCL], F32, tag="part")
                        cur = D
                        while cur > 2:
                            hh = cur // 2
                            nc.vector.tensor_add(
                                msg[:, 0:hh, :], msg[:, 0:hh, :],
                                msg[:, cur - hh:cur, :])
                            cur -= hh
                        if cur == 2:
                            nc.vector.tensor_add(part[:, :], msg[:, 0, :],
                                                 msg[:, 1, :])
                        else:
                            nc.vector.tensor_copy(part[:, :], msg[:, 0, :])
                        parts.append(part)
                    unn = spool.tile([128, HCL], F32, tag="unn")
                    nc.vector.tensor_add(unn[:, :], parts[0][:, :],
                                         parts[1][:, :])
                    den = spool.tile([128, NH], F32, tag="dent")
                    nc.vector.tensor_add(den[:, :], dens[0][:, :],
                                         dens[1][:, :])
                    # den >= exp(leakyrelu(self-loop logit)) > 0: every dst
                    # has a self-loop and |e| is O(1), so no eps guard needed
                    rec = spool.tile([128, NH], F32, tag="rec")
                    nc.vector.reciprocal(rec[:, :], den[:, :])
                    out_cb(t, unn, rec)

        # ---- L1 epilogue: normalize, +b1, ELU, store h1 ----
        def l1_out(t, unn, rec):
            y = spool.tile([128, HC1], F32, tag="y1")
            nc.vector.tensor_mul(
                y.rearrange("p (h c) -> p h c", h=H),
                unn.rearrange("p (h c) -> p h c", h=H),
                _bc(rec[:, :].unsqueeze(2), (128, H, HID)))
            nc.vector.tensor_add(y[:, :], y[:, :], B1_s[:, :])
            mn = spool.tile([128, HC1], F32, tag="mn1")
            nc.vector.tensor_scalar_min(mn[:, :], y[:, :], 0.0)
            nc.vector.tensor_scalar_max(y[:, :], y[:, :], 0.0)
            em = spool.tile([128, HC1], F32, tag="em1")
            nc.scalar.activation(em[:, :], mn[:, :], Act.Exp)
            h1t = spool.tile([128, HC1], BF16, tag="h1t")
            nc.vector.scalar_tensor_tensor(h1t[:, :], em[:, :], -1.0, y[:, :],
                                           op0=Alu.add, op1=Alu.add)
            nc.sync.dma_start(h1d[t * 128:(t + 1) * 128, :], h1t[:, :])

        edge_layer(1, table1, ROW1, HID, H, lambda t: arL[:, t, :], l1_out)

        # ---- L2 projection from h1 (DMA-transpose h1d) ----
        h1T = epool.tile([128, KC, NT], BF16, tag="h1T")
        for k in range(KC):
            nc.sync.dma_start_transpose(h1T[:, k, :],
                                        h1d[:, k * 128:(k + 1) * 128])
        tstage2 = epool.tile([128, TS_T, ROW2], BF16, tag="tstage2")
        nc.vector.memset(tstage2[:, :, :], 0.0)
        for t in range(T):
            ps = psum.tile([128, OUT + 2], F32, tag="proj2")
            for k in range(KC):
                nc.tensor.matmul(ps[:, :], h1T[:, k, t * 128:(t + 1) * 128],
                                 RHS2[:, k, :], start=(k == 0),
                                 stop=(k == KC - 1))
            nc.scalar.copy(tstage2[:, t, 0:OUT], ps[:, 0:OUT])
            al2_view = tstage2[:, t, OUT:OUT + 2].bitcast(F32)
            nc.vector.tensor_copy(al2_view[:, :], ps[:, OUT:OUT + 1])
            nc.vector.tensor_copy(ar2L[:, t, :], ps[:, OUT + 1:OUT + 2])
        sent2 = epool.tile([1, ROW2], BF16, tag="sent2")
        nc.vector.memset(sent2[:, :], 0.0)
        nc.vector.memset(sent2[:, OUT:OUT + 2].bitcast(F32), AL_SENT)
        nc.sync.dma_start(chunk2[0:1, :], sent2[:, :])
        nc.sync.dma_start(
            chunk2[1:1 + full_t * 128, 0:OUT + 2].rearrange(
                "(t p) c -> p t c", p=128),
            tstage2[:, 0:full_t, 0:OUT + 2])
        if rem > 0:
            nc.sync.dma_start(chunk2[1 + full_t * 128:CHUNK, 0:OUT + 2],
                              tstage2[0:rem, full_t, 0:OUT + 2])
        nc.gpsimd.collective_compute(
            "AllGather", Alu.bypass,
            replica_groups=[list(range(cfg.R))],
            ins=[chunk2[:, :].opt()], outs=[table2[:, :].opt()])

        # ---- L2 epilogue: normalize, +b2, log_softmax, store out ----
        ostage = epool.tile([128, T, OUT], F32, tag="ostage")

        def l2_out(t, unn, rec):
            y = spool.tile([128, OUT], F32, tag="y2")
            nc.vector.tensor_scalar_mul(y[:, :], unn[:, :], rec[:, 0:1])
            nc.vector.tensor_add(y[:, :], y[:, :], B2_s[:, :])
            # |y| is O(10): exp stays finite in f32, no max-subtract needed
            ex = spool.tile([128, OUT], F32, tag="ex2")
            ssum = spool.tile([128, 1], F32, tag="ss2")
            nc.scalar.activation(ex[:, :], y[:, :], Act.Exp,
                                 accum_out=ssum[:, :])
            ls = spool.tile([128, 1], F32, tag="ls2")
            nc.scalar.activation(ls[:, :], ssum[:, :], Act.Ln)
            nc.vector.tensor_scalar_sub(ostage[:, t, :], y[:, :], ls[:, 0:1])

        edge_layer(2, table2, ROW2, OUT, 1, lambda t: ar2L[:, t, :], l2_out)
        nc.sync.dma_start(out_d.ap().rearrange("(t p) c -> p t c", p=128),
                          ostage[:, :, :])

    nc.compile()
    return nc


def _host_inputs(cfg: Cfg, sch: Sched, inputs: dict):
    """Build per-rank in_maps from the full problem inputs."""
    x = np.asarray(inputs["x"], np.float32)
    W1 = np.asarray(inputs["W1"], np.float32)
    a1_src = np.asarray(inputs["a1_src"], np.float32)
    a1_dst = np.asarray(inputs["a1_dst"], np.float32)
    b1 = np.asarray(inputs["b1"], np.float32)
    W2 = np.asarray(inputs["W2"], np.float32)
    a2_src = np.asarray(inputs["a2_src"], np.float32)
    a2_dst = np.asarray(inputs["a2_dst"], np.float32)
    b2 = np.asarray(inputs["b2"], np.float32)
    H, HID, HC1, OUT = cfg.HEADS, cfg.HID, cfg.HC1, cfg.OUT

    # block-diagonal per-head attention matrices: al = h @ A1s
    A1s = np.zeros((HC1, H), np.float32)
    A1d = np.zeros((HC1, H), np.float32)
    for h in range(H):
        A1s[h * HID:(h + 1) * HID, h] = a1_src[h]
        A1d[h * HID:(h + 1) * HID, h] = a1_dst[h]

    common = {
        "W1": np.ascontiguousarray(W1),
        "W1T": np.ascontiguousarray(W1.T),
        "A1s": A1s, "A1d": A1d,
        "B1rep": np.tile(b1[None, :], (128, 1)).astype(np.float32),
        "W2": np.ascontiguousarray(W2),
        "W2T": np.ascontiguousarray(W2.T),
        "a2s": np.ascontiguousarray(a2_src.reshape(OUT, 1)),
        "a2d": np.ascontiguousarray(a2_dst.reshape(OUT, 1)),
        "B2rep": np.tile(b2[None, :], (128, 1)).astype(np.float32),
    }
    in_maps = []
    for r in range(cfg.R):
        m = dict(common)
        xp = np.zeros((cfg.T * 128, x.shape[1]), np.float32)
        xp[:cfg.NPR] = x[sch.perm[r]]
        m["xT"] = np.ascontiguousarray(xp.T)
        m["idxs"] = np.ascontiguousarray(sch.idx16[r])
        in_maps.append(m)
    return in_maps


def run(cfg: Cfg, inputs: dict, trace: bool = False):
    edge_index = np.asarray(inputs["edge_index"])
    loops = np.arange(cfg.N, dtype=edge_index.dtype)
    src = np.concatenate([edge_index[0], loops]).astype(np.int64)
    dst = np.concatenate([edge_index[1], loops]).astype(np.int64)

    sch = build_schedule(cfg, src, dst)
    nc = build_program(cfg, sch)
    in_maps = _host_inputs(cfg, sch, inputs)
    res = bass_utils.run_bass_kernel_spmd(
        nc, in_maps, core_ids=list(range(cfg.R)), trace=trace)
    out = np.empty((cfg.N, cfg.OUT), np.float32)
    for r in range(cfg.R):
        o = res.results[r]["out"]
        out[sch.perm[r]] = o[:cfg.NPR]
    return out, res


def kernel(**inputs) -> np.ndarray:
    cfg = Cfg()
    out, _ = run(cfg, inputs)
    return out


if __name__ == "__main__":
    import reference
    inputs = {k: np.asarray(v) for k, v in reference.setup_inputs().items()}
    out = kernel(**inputs)
    exp = np.asarray(reference.reference(**reference.setup_inputs()))
    err = np.abs(out - exp).max() / (np.abs(exp).max() + 1e-12)
    print("rel err:", err)


# revision 35
# speedup vs baseline: 1.0403x; 1.0214x over previous
"""2-layer GAT (gnn_message_passing) on 8 TRN2 NeuronCores.

Strategy (graph/data parallel, per sharding hint):
  - Nodes are partitioned across 8 ranks (6250 dst nodes each). Each rank owns
    the segment-softmax + aggregation for its destination nodes.
  - Per layer, every rank computes the projected features (h = x @ W,
    attention source/dest logits al/ar fused into the same matmul via an
    augmented RHS) for ITS OWN nodes, writes them as rows of a gather table
    (768B rows for layer 1: 256 bf16 h + 8 f32 al; 256B rows for layer 2),
    then an AllGather replicates the full table to every rank.
  - Edge stage: destinations are degree-sorted and packed into tiles of 128
    (dst on partitions); each dst gets a padded run of incoming-edge "slots"
    along the free dimension. Source rows are fetched with dma_gather
    (SWDGE indexed gather). Since gather indices are int16, the table is
    addressed through two base windows (rows [0,32768) and
    [TROWS-32768, TROWS)), and each dst's slots are split into a "lo" and
    "hi" range accordingly. Tiles are greedily grouped (up to SLOT_CAP
    slots) into shared gather calls, and the lo/hi gathers of consecutive
    groups rotate over 4 SWDGE queues so their drains overlap.
  - Slot-grid math per (tile, region): e = leakyrelu(al_src + ar_dst),
    p = exp(e) (no max-subtract needed at these magnitudes), denom =
    free-dim reduce, normalization applied AFTER aggregation.
  - Aggregation: msg = h_src * p (p broadcast over channels via a step-0
    AP), then a pairwise tree of wide tensor adds along the slot dim.
  - Padding slots read a sentinel table row (h = 0, al = -1e30 -> p = 0).

The full output is assembled on the host from the 8 per-rank outputs
(undoing the degree-sort permutation).
"""

import sys
from contextlib import ExitStack
from dataclasses import dataclass

import numpy as np

for _p in ("/opt/trn_rl_repo",):
    if _p not in sys.path:
        sys.path.insert(0, _p)

import concourse.bass as bass
import concourse.bacc as bacc
import concourse.mybir as mybir
import concourse.tile as tile
from concourse import bass_utils

F32 = mybir.dt.float32
BF16 = mybir.dt.bfloat16
I16 = mybir.dt.int16
AL_SENT = -1.0e30
Alu = mybir.AluOpType
Act = mybir.ActivationFunctionType


@dataclass
class Cfg:
    N: int = 50000
    E: int = 500000          # edges before self-loops
    F_IN: int = 128
    HID: int = 32
    HEADS: int = 8
    OUT: int = 64
    NEG: float = 0.2
    R: int = 8
    SLOT_CAP: int = 32       # max slots per gather group (SBUF budget)
    hi_base: int = -1        # -1: auto (TROWS - 32768, clamped to >= 0)

    @property
    def HC1(self):
        return self.HEADS * self.HID     # 256

    @property
    def NPR(self):
        return self.N // self.R

    @property
    def CHUNK(self):
        return self.NPR + 1              # + sentinel row

    @property
    def TROWS(self):
        return self.R * self.CHUNK

    @property
    def T(self):
        return (self.NPR + 127) // 128   # dst tiles per rank

    @property
    def ROW1(self):
        return 384                       # bf16 elems: 256 h + 16 (8xf32 al) + pad

    @property
    def ROW2(self):
        return 128                       # bf16 elems: 64 h2 + 2 (1xf32 al2) + pad

    @property
    def HI_BASE(self):
        if self.hi_base >= 0:
            return self.hi_base
        return max(0, self.TROWS - 32768)

    @property
    def LO_LIM(self):
        # rows addressable from base 0
        return min(self.TROWS, 32768)


@dataclass
class Sched:
    perm: np.ndarray          # [R, NPR] perm[r][pos] = global node id
    sortpos: np.ndarray       # [N] position of node within its rank
    D_lo: np.ndarray          # [T]
    D_hi: np.ndarray          # [T]
    groups: list              # list of (t0, t1) tile ranges
    idx16: np.ndarray         # [R, 128, TOTCOL] int16
    call_cols: list           # per group: (lo_col0, lo_ncol, hi_col0, hi_ncol)
    sub_off: np.ndarray       # [T, 2] slot offset of (tile, region) in group buffer
    group_of: np.ndarray      # [T] group index of tile


def build_schedule(cfg: Cfg, src: np.ndarray, dst: np.ndarray) -> Sched:
    N, R, NPR, CHUNK, T = cfg.N, cfg.R, cfg.NPR, cfg.CHUNK, cfg.T
    deg = np.bincount(dst, minlength=N).astype(np.int64)
    odeg = np.bincount(src, minlength=N).astype(np.int64)

    # assign the highest out-degree nodes to the ranks whose table chunks sit
    # in the lo/hi window overlap, maximizing the flexible-edge fraction
    oorder = np.argsort(-odeg, kind="stable")
    fill_order = [3, 4, 2, 5, 1, 6, 0, 7]
    rank_of = np.empty(N, np.int64)
    for i, r in enumerate(fill_order):
        rank_of[oorder[i * NPR:(i + 1) * NPR]] = r

    sortpos = np.empty(N, np.int64)
    perm = np.empty((R, NPR), np.int64)
    for r in range(R):
        nodes = np.where(rank_of == r)[0]
        order = np.argsort(-deg[nodes], kind="stable")
        perm[r] = nodes[order]
        sortpos[perm[r]] = np.arange(NPR)
    # chunk row 0 of every rank is its sentinel row; real rows start at 1
    row_of = rank_of * CHUNK + 1 + sortpos        # [N] table row of each node

    src_row = row_of[src]
    # categories: 0 = forced lo, 1 = flexible, 2 = forced hi
    cat = np.where(src_row < cfg.HI_BASE, 0, np.where(src_row < cfg.LO_LIM, 1, 2))

    # global dst key: (rank, sorted position)
    dkey = rank_of[dst] * NPR + sortpos[dst]
    order = np.lexsort((cat, dkey))
    s_src_row = src_row[order]
    s_dkey = dkey[order]

    cnt = np.bincount(dkey, minlength=R * NPR)
    cnt_lo = np.bincount(dkey[cat == 0], minlength=R * NPR)
    cnt_hi = np.bincount(dkey[cat == 2], minlength=R * NPR)
    start = np.concatenate([[0], np.cumsum(cnt)])[:-1]

    # per-dst lo count: balance towards half, respecting forced counts
    half = (cnt + 1) // 2
    nlo = np.clip(half, cnt_lo, cnt - cnt_hi)
    nhi = cnt - nlo

    pos_in_dst = np.arange(len(order)) - start[s_dkey]
    is_lo = pos_in_dst < nlo[s_dkey]
    slot = np.where(is_lo, pos_in_dst, pos_in_dst - nlo[s_dkey])

    # slot grid shared by all ranks
    D_lo = np.zeros(T, np.int64)
    D_hi = np.zeros(T, np.int64)
    nlo_g = nlo.reshape(R, NPR)
    nhi_g = nhi.reshape(R, NPR)
    for t in range(T):
        sl = slice(t * 128, min((t + 1) * 128, NPR))
        D_lo[t] = max(1, nlo_g[:, sl].max())
        D_hi[t] = max(1, nhi_g[:, sl].max())
    assert (D_lo + D_hi).max() <= cfg.SLOT_CAP, (
        f"tile needs {(D_lo + D_hi).max()} slots > SLOT_CAP {cfg.SLOT_CAP}")

    # greedy grouping of tiles, capped at SLOT_CAP slots
    groups = []
    group_of = np.zeros(T, np.int64)
    t0 = 0
    while t0 < T:
        t1 = t0 + 1
        tot = D_lo[t0] + D_hi[t0]
        while t1 < T and tot + D_lo[t1] + D_hi[t1] <= cfg.SLOT_CAP:
            tot += D_lo[t1] + D_hi[t1]
            t1 += 1
        group_of[t0:t1] = len(groups)
        groups.append((t0, t1))
        t0 = t1

    # slot offsets of each (tile, region) within its group buffer:
    # [lo slots of t0 | lo t1 | ... | hi t0 | hi t1 | ...]
    sub_off = np.zeros((T, 2), np.int64)
    call_cols = []
    col = 0
    pos_base_lo = np.zeros(T, np.int64)
    pos_base_hi = np.zeros(T, np.int64)
    for (t0, t1) in groups:
        S_lo = int(D_lo[t0:t1].sum())
        S_hi = int(D_hi[t0:t1].sum())
        off = 0
        lo_col0 = col
        for t in range(t0, t1):
            sub_off[t, 0] = off
            pos_base_lo[t] = col * 16 + off * 128
            off += D_lo[t]
        col += S_lo * 8  # 128/16 columns per slot-column
        hi_col0 = col
        off2 = 0
        for t in range(t0, t1):
            sub_off[t, 1] = S_lo + off2
            pos_base_hi[t] = col * 16 + off2 * 128
            off2 += D_hi[t]
        col += S_hi * 8
        call_cols.append((lo_col0, S_lo * 8, hi_col0, S_hi * 8))
    TOTCOL = col
    TOTPOS = TOTCOL * 16

    SENT_LO = 0                                    # rank 0 sentinel row
    SENT_HI = (R - 1) * CHUNK - cfg.HI_BASE        # last rank sentinel, local
    assert 0 <= SENT_HI < 32768

    # fill idx values per rank
    e_rank = s_dkey // NPR
    e_pos = s_dkey % NPR
    e_tile = e_pos // 128
    e_part = e_pos % 128
    idx16 = np.empty((R, 128, TOTCOL), np.int16)
    for r in range(R):
        vals = np.empty(TOTPOS, np.int32)
        for (t0, t1), (lc0, lnc, hc0, hnc) in zip(groups, call_cols):
            vals[lc0 * 16:(lc0 + lnc) * 16] = SENT_LO
            vals[hc0 * 16:(hc0 + hnc) * 16] = SENT_HI
        m = (e_rank == r)
        mlo = m & is_lo
        mhi = m & ~is_lo
        p_lo = pos_base_lo[e_tile[mlo]] + slot[mlo] * 128 + e_part[mlo]
        vals[p_lo] = s_src_row[mlo]
        p_hi = pos_base_hi[e_tile[mhi]] + slot[mhi] * 128 + e_part[mhi]
        vals[p_hi] = s_src_row[mhi] - cfg.HI_BASE
        assert vals.min() >= 0 and vals.max() < 32768
        idx16[r] = np.tile(vals.reshape(-1, 16).T, (8, 1))

    return Sched(perm=perm, sortpos=sortpos, D_lo=D_lo, D_hi=D_hi,
                 groups=groups, idx16=idx16, call_cols=call_cols,
                 sub_off=sub_off, group_of=group_of)


def _bc(ap, shape):
    """broadcast an AP to shape (step-0 dims)"""
    return ap.broadcast_to(list(shape))


def build_program(cfg: Cfg, sch: Sched):
    """Build the single SPMD Bass program. Returns nc."""
    nc = bacc.Bacc("TRN2", target_bir_lowering=False, debug=False,
                   num_devices=cfg.R, num_swdge_queues=4)
    T, NPR, CHUNK, TROWS = cfg.T, cfg.NPR, cfg.CHUNK, cfg.TROWS
    HC1, H, HID, OUT = cfg.HC1, cfg.HEADS, cfg.HID, cfg.OUT
    ROW1, ROW2 = cfg.ROW1, cfg.ROW2
    TOTCOL = sch.idx16.shape[2]
    NT = T * 128

    # ---- I/O ----
    xT = nc.dram_tensor("xT", [cfg.F_IN, NT], F32, kind="ExternalInput")
    idxs_d = nc.dram_tensor("idxs", [128, TOTCOL], I16, kind="ExternalInput")
    W1_d = nc.dram_tensor("W1", [cfg.F_IN, HC1], F32, kind="ExternalInput")
    W1T_d = nc.dram_tensor("W1T", [HC1, cfg.F_IN], F32, kind="ExternalInput")
    A1s_d = nc.dram_tensor("A1s", [HC1, H], F32, kind="ExternalInput")
    A1d_d = nc.dram_tensor("A1d", [HC1, H], F32, kind="ExternalInput")
    B1_d = nc.dram_tensor("B1rep", [128, HC1], F32, kind="ExternalInput")
    W2_d = nc.dram_tensor("W2", [HC1, OUT], F32, kind="ExternalInput")
    W2T_d = nc.dram_tensor("W2T", [OUT, HC1], F32, kind="ExternalInput")
    a2s_d = nc.dram_tensor("a2s", [OUT, 1], F32, kind="ExternalInput")
    a2d_d = nc.dram_tensor("a2d", [OUT, 1], F32, kind="ExternalInput")
    B2_d = nc.dram_tensor("B2rep", [128, OUT], F32, kind="ExternalInput")
    out_d = nc.dram_tensor("out", [NT, OUT], F32, kind="ExternalOutput")

    KC = HC1 // 128   # contraction chunks over HC1 (2)

    with tile.TileContext(nc) as tc, ExitStack() as ctx:
        dram = ctx.enter_context(tc.tile_pool(name="dram", bufs=1, space="DRAM"))
        const = ctx.enter_context(tc.tile_pool(name="const", bufs=1))
        psum = ctx.enter_context(tc.tile_pool(name="psum", bufs=2, space="PSUM"))

        # DRAM scratch
        chunk1 = dram.tile([CHUNK, ROW1], BF16)
        table1 = dram.tile([TROWS, ROW1], BF16, addr_space="Shared")
        chunk2 = dram.tile([CHUNK, ROW2], BF16)
        table2 = dram.tile([TROWS, ROW2], BF16, addr_space="Shared")
        h1d = dram.tile([NT, HC1], BF16)

        # ---- persistent constants ----
        idx_s = const.tile([128, TOTCOL], I16, tag="idx")
        nc.sync.dma_start(idx_s[:, :], idxs_d[:, :])
        RHS1 = const.tile([128, HC1 + 2 * H], F32, tag="rhs1")
        nc.sync.dma_start(RHS1[:, 0:HC1], W1_d[:, :])
        B1_s = const.tile([128, HC1], F32, tag="b1")
        nc.sync.dma_start(B1_s[:, :], B1_d[:, :])
        B2_s = const.tile([128, OUT], F32, tag="b2")
        nc.sync.dma_start(B2_s[:, :], B2_d[:, :])
        RHS2 = const.tile([128, KC, OUT + 2], BF16, tag="rhs2")
        nc.gpsimd.dma_start(RHS2[:, :, 0:OUT],
                            W2_d.ap().rearrange("(k p) c -> p k c", p=128))
        arL = const.tile([128, T, H], F32, tag="arL")
        nc.vector.memset(arL[:, :, :], 0.0)
        ar2L = const.tile([128, T, 1], F32, tag="ar2L")
        nc.vector.memset(ar2L[:, :, :], 0.0)

        # ================= phase 1: projection + table 1 ====================
        TS_T = (NPR + 1 + 127) // 128
        full_t = NPR // 128
        rem = NPR - full_t * 128
        with tc.tile_pool(name="ph1", bufs=1) as ph1:
            xT_s = ph1.tile([128, NT], F32, tag="xT")
            nc.sync.dma_start(xT_s[:, :], xT[:, :])
            W1T_s = ph1.tile([128, KC, 128], F32, tag="w1t")
            nc.sync.dma_start(W1T_s[:, :, :],
                              W1T_d.ap().rearrange("(k p) f -> p k f", p=128))
            A1s_s = ph1.tile([128, KC, H], F32, tag="a1s")
            nc.sync.dma_start(A1s_s[:, :, :],
                              A1s_d.ap().rearrange("(k p) h -> p k h", p=128))
            A1d_s = ph1.tile([128, KC, H], F32, tag="a1d")
            nc.sync.dma_start(A1d_s[:, :, :],
                              A1d_d.ap().rearrange("(k p) h -> p k h", p=128))
            W2T_s = ph1.tile([OUT, HC1], F32, tag="w2t")
            nc.sync.dma_start(W2T_s[:, :], W2T_d[:, :])
            a2s_s = ph1.tile([OUT, 1], F32, tag="a2s")
            nc.sync.dma_start(a2s_s[:, :], a2s_d[:, :])
            a2d_s = ph1.tile([OUT, 1], F32, tag="a2d")
            nc.sync.dma_start(a2d_s[:, :], a2d_d[:, :])

            # fold attention vectors into projection RHS
            for (dst_off, A_s) in ((HC1, A1s_s), (HC1 + H, A1d_s)):
                ps = psum.tile([128, H], F32, tag="wprep")
                for k in range(KC):
                    nc.tensor.matmul(ps[:, :], W1T_s[:, k, :], A_s[:, k, :],
                                     start=(k == 0), stop=(k == KC - 1))
                nc.vector.tensor_copy(RHS1[:, dst_off:dst_off + H], ps[:, :])
            for (dst_off, a_s) in ((OUT, a2s_s), (OUT + 1, a2d_s)):
                for k in range(KC):
                    ps = psum.tile([128, 1], F32, tag="wprep2")
                    nc.tensor.matmul(ps[:, :], W2T_s[:, k * 128:(k + 1) * 128],
                                     a_s[:, :], start=True, stop=True)
                    nc.vector.tensor_copy(RHS2[:, k, dst_off:dst_off + 1],
                                          ps[:, :])

            tstage = ph1.tile([128, TS_T, ROW1], BF16, tag="tstage1")
            nc.vector.memset(tstage[:, :, :], 0.0)
            for t in range(T):
                ps = psum.tile([128, HC1 + 2 * H], F32, tag="proj1")
                nc.tensor.matmul(ps[:, :], xT_s[:, t * 128:(t + 1) * 128],
                                 RHS1[:, :], start=True, stop=True)
                nc.scalar.copy(tstage[:, t, 0:HC1], ps[:, 0:HC1])
                al_view = tstage[:, t, HC1:HC1 + 2 * H].bitcast(F32)
                nc.vector.tensor_copy(al_view[:, :], ps[:, HC1:HC1 + H])
                nc.vector.tensor_copy(arL[:, t, :],
                                      ps[:, HC1 + H:HC1 + 2 * H])
            # sentinel row -> chunk row 0 (h = 0, al = -1e30)
            sent1 = ph1.tile([1, ROW1], BF16, tag="sent1")
            nc.vector.memset(sent1[:, :], 0.0)
            nc.vector.memset(sent1[:, HC1:HC1 + 2 * H].bitcast(F32), AL_SENT)
            nc.sync.dma_start(chunk1[0:1, :], sent1[:, :])
            nc.sync.dma_start(
                chunk1[1:1 + full_t * 128, 0:HC1 + 2 * H].rearrange(
                    "(t p) c -> p t c", p=128),
                tstage[:, 0:full_t, 0:HC1 + 2 * H])
            if rem > 0:
                nc.sync.dma_start(chunk1[1 + full_t * 128:CHUNK, 0:HC1 + 2 * H],
                                  tstage[0:rem, full_t, 0:HC1 + 2 * H])
        nc.gpsimd.collective_compute(
            "AllGather", Alu.bypass,
            replica_groups=[list(range(cfg.R))],
            ins=[chunk1[:, :].opt()], outs=[table1[:, :].opt()])

        epool = ctx.enter_context(tc.tile_pool(name="edge", bufs=1))
        gpool = ctx.enter_context(tc.tile_pool(name="gpool", bufs=3))
        spool = ctx.enter_context(tc.tile_pool(name="spool", bufs=3))
        ppool = ctx.enter_context(tc.tile_pool(name="ppool", bufs=2))

        # ================= edge phase (shared for both layers) ==============
        def edge_layer(layer, table, ROW, CH, NH, arl_ap, out_cb):
            """layer: 1 or 2. CH: channels per head (32 / 64). NH: heads.
            arl_ap(t) -> [128, NH] f32 AP; out_cb(t, unn, rec) emits epilogue.
            """
            HCL = CH * NH
            lo_tab = table[0:cfg.LO_LIM, :]
            hi_tab = table[cfg.HI_BASE:TROWS, :]
            for gi, ((t0, t1), (lc0, lnc, hc0, hnc)) in enumerate(
                    zip(sch.groups, sch.call_cols)):
                S_lo = int(sch.D_lo[t0:t1].sum())
                S_hi = int(sch.D_hi[t0:t1].sum())
                S = S_lo + S_hi
                g = gpool.tile([128, cfg.SLOT_CAP, ROW], BF16, tag="gbuf")
                nc.gpsimd.dma_gather(
                    g[:, 0:S_lo, :], lo_tab, idx_s[:, lc0:lc0 + lnc],
                    num_idxs=S_lo * 128, num_idxs_reg=S_lo * 128,
                    elem_size=ROW, elem_step=ROW, single_packet=False,
                    queue_num=(2 * gi) % 4)
                nc.gpsimd.dma_gather(
                    g[:, S_lo:S, :], hi_tab, idx_s[:, hc0:hc0 + hnc],
                    num_idxs=S_hi * 128, num_idxs_reg=S_hi * 128,
                    elem_size=ROW, elem_step=ROW, single_packet=False,
                    queue_num=(2 * gi + 1) % 4)

                for t in range(t0, t1):
                    parts, dens = [], []
                    for reg in (0, 1):
                        D = int((sch.D_lo, sch.D_hi)[reg][t])
                        so = int(sch.sub_off[t, reg])
                        gs = g[:, so:so + D, :]
                        # e = leakyrelu(al + ar)
                        e = spool.tile([128, cfg.SLOT_CAP, NH], F32, tag="e")
                        al = gs[:, :, HCL:HCL + 2 * NH].bitcast(F32)
                        nc.vector.tensor_add(
                            e[:, 0:D, :], al,
                            _bc(arl_ap(t).unsqueeze(1), (128, D, NH)))
                        nc.vector.scalar_tensor_tensor(
                            e[:, 0:D, :], e[:, 0:D, :], cfg.NEG, e[:, 0:D, :],
                            op0=Alu.mult, op1=Alu.max)
                        # p = exp(e)  (bf16 out)
                        p = spool.tile([128, cfg.SLOT_CAP, NH], BF16, tag="p")
                        nc.scalar.activation(p[:, 0:D, :], e[:, 0:D, :],
                                             Act.Exp)
                        # denom partial = sum over slots
                        den = spool.tile([128, NH], F32, tag="den")
                        nc.vector.tensor_reduce(
                            den[:, :], p[:, 0:D, :].transpose([0, 2, 1]),
                            axis=mybir.AxisListType.X, op=Alu.add)
                        dens.append(den)
                        # msg = h * p (p broadcast over channels, step-0 AP)
                        msg = ppool.tile([128, cfg.SLOT_CAP, NH, CH], BF16,
                                         tag="msg")
                        pb = p[:, 0:D, :].unsqueeze(3).broadcast_to(
                            [128, D, NH, CH])
                        nc.vector.tensor_mul(
                            msg[:, 0:D, :, :],
                            gs[:, :, 0:HCL].rearrange("p d (h c) -> p d h c",
                                                      h=NH),
                            pb)
                        msg = msg[:, :, :, :].rearrange("p d h c -> p d (h c)")
                        # tree-sum over slots -> part [128, HCL] f32
                        part = spool.tile([128, HCL], F32, tag="part")
                        cur = D
                        while cur > 2:
                            hh = cur // 2
                            nc.vector.tensor_add(
                                msg[:, 0:hh, :], msg[:, 0:hh, :],
                                msg[:, cur - hh:cur, :])
                            cur -= hh
                        if cur == 2:
                            nc.vector.tensor_add(part[:, :], msg[:, 0, :],
                                                 msg[:, 1, :])
                        else:
                            nc.vector.tensor_copy(part[:, :], msg[:, 0, :])
                        parts.append(part)
                    unn = spool.tile([128, HCL], F32, tag="unn")
                    nc.vector.tensor_add(unn[:, :], parts[0][:, :],
                                         parts[1][:, :])
                    den = spool.tile([128, NH], F32, tag="dent")
                    nc.vector.tensor_add(den[:, :], dens[0][:, :],
                                         dens[1][:, :])
                    # den >= exp(leakyrelu(self-loop logit)) > 0: every dst
                    # has a self-loop and |e| is O(1), so no eps guard needed
                    rec = spool.tile([128, NH], F32, tag="rec")
                    nc.vector.reciprocal(rec[:, :], den[:, :])
                    out_cb(t, unn, rec)

        # L2 projection in 4 batches hidden under the L1 edge phase; each
        # batch has its own transpose target so xbar writes start at offset 0
        BATCHES = [(0, 13), (13, 26), (26, 39), (39, T)]
        h1T_b = []
        for _i, (_b0, _b1) in enumerate(BATCHES):
            h1T_bt = epool.tile([128, KC, 128 * (_b1 - _b0)], BF16,
                                tag=f"h1T{_i}")
            h1T_b.append(h1T_bt)
        tstage2 = epool.tile([128, TS_T, ROW2], BF16, tag="tstage2")
        nc.vector.memset(tstage2[:, :, :], 0.0)

        def proj2_batch(bi):
            b0, b1 = BATCHES[bi]
            ht = h1T_b[bi]
            for k in range(KC):
                nc.sync.dma_start_transpose(
                    ht[:, k, :],
                    h1d[b0 * 128:b1 * 128, k * 128:(k + 1) * 128])
            for t in range(b0, b1):
                o = (t - b0) * 128
                ps = psum.tile([128, OUT + 2], F32, tag="proj2")
                for k in range(KC):
                    nc.tensor.matmul(ps[:, :], ht[:, k, o:o + 128],
                                     RHS2[:, k, :], start=(k == 0),
                                     stop=(k == KC - 1))
                nc.scalar.copy(tstage2[:, t, 0:OUT], ps[:, 0:OUT])
                al2_view = tstage2[:, t, OUT:OUT + 2].bitcast(F32)
                nc.vector.tensor_copy(al2_view[:, :], ps[:, OUT:OUT + 1])
                nc.vector.tensor_copy(ar2L[:, t, :], ps[:, OUT + 1:OUT + 2])

        # ---- L1 epilogue: normalize, +b1, ELU, store h1 ----
        def l1_out(t, unn, rec):
            y = spool.tile([128, HC1], F32, tag="y1")
            nc.vector.tensor_mul(
                y.rearrange("p (h c) -> p h c", h=H),
                unn.rearrange("p (h c) -> p h c", h=H),
                _bc(rec[:, :].unsqueeze(2), (128, H, HID)))
            nc.vector.tensor_add(y[:, :], y[:, :], B1_s[:, :])
            mn = spool.tile([128, HC1], F32, tag="mn1")
            nc.vector.tensor_scalar_min(mn[:, :], y[:, :], 0.0)
            nc.vector.tensor_scalar_max(y[:, :], y[:, :], 0.0)
            em = spool.tile([128, HC1], F32, tag="em1")
            nc.scalar.activation(em[:, :], mn[:, :], Act.Exp)
            h1t = spool.tile([128, HC1], BF16, tag="h1t")
            nc.vector.scalar_tensor_tensor(h1t[:, :], em[:, :], -1.0, y[:, :],
                                           op0=Alu.add, op1=Alu.add)
            nc.sync.dma_start(h1d[t * 128:(t + 1) * 128, :], h1t[:, :])
            if t + 1 == 13:
                proj2_batch(0)
            elif t + 1 == 26:
                proj2_batch(1)
            elif t + 1 == 39:
                proj2_batch(2)

        edge_layer(1, table1, ROW1, HID, H, lambda t: arL[:, t, :], l1_out)
        proj2_batch(3)
        sent2 = epool.tile([1, ROW2], BF16, tag="sent2")
        nc.vector.memset(sent2[:, :], 0.0)
        nc.vector.memset(sent2[:, OUT:OUT + 2].bitcast(F32), AL_SENT)
        nc.sync.dma_start(chunk2[0:1, :], sent2[:, :])
        nc.sync.dma_start(
            chunk2[1:1 + full_t * 128, 0:OUT + 2].rearrange(
                "(t p) c -> p t c", p=128),
            tstage2[:, 0:full_t, 0:OUT + 2])
        if rem > 0:
            nc.sync.dma_start(chunk2[1 + full_t * 128:CHUNK, 0:OUT + 2],
                              tstage2[0:rem, full_t, 0:OUT + 2])
        nc.gpsimd.collective_compute(
            "AllGather", Alu.bypass,
            replica_groups=[list(range(cfg.R))],
            ins=[chunk2[:, :].opt()], outs=[table2[:, :].opt()])

        # ---- L2 epilogue: normalize, +b2, log_softmax, store out ----
        ostage = epool.tile([128, T, OUT], F32, tag="ostage")

        def l2_out(t, unn, rec):
            y = spool.tile([128, OUT], F32, tag="y2")
            nc.vector.tensor_scalar_mul(y[:, :], unn[:, :], rec[:, 0:1])
            nc.vector.tensor_add(y[:, :], y[:, :], B2_s[:, :])
            # |y| is O(10): exp stays finite in f32, no max-subtract needed
            ex = spool.tile([128, OUT], F32, tag="ex2")
            ssum = spool.tile([128, 1], F32, tag="ss2")
            nc.scalar.activation(ex[:, :], y[:, :], Act.Exp,
                                 accum_out=ssum[:, :])
            ls = spool.tile([128, 1], F32, tag="ls2")
            nc.scalar.activation(ls[:, :], ssum[:, :], Act.Ln)
            nc.vector.tensor_scalar_sub(ostage[:, t, :], y[:, :], ls[:, 0:1])

        edge_layer(2, table2, ROW2, OUT, 1, lambda t: ar2L[:, t, :], l2_out)
        nc.sync.dma_start(out_d.ap().rearrange("(t p) c -> p t c", p=128),
                          ostage[:, :, :])

    nc.compile()
    return nc


def _host_inputs(cfg: Cfg, sch: Sched, inputs: dict):
    """Build per-rank in_maps from the full problem inputs."""
    x = np.asarray(inputs["x"], np.float32)
    W1 = np.asarray(inputs["W1"], np.float32)
    a1_src = np.asarray(inputs["a1_src"], np.float32)
    a1_dst = np.asarray(inputs["a1_dst"], np.float32)
    b1 = np.asarray(inputs["b1"], np.float32)
    W2 = np.asarray(inputs["W2"], np.float32)
    a2_src = np.asarray(inputs["a2_src"], np.float32)
    a2_dst = np.asarray(inputs["a2_dst"], np.float32)
    b2 = np.asarray(inputs["b2"], np.float32)
    H, HID, HC1, OUT = cfg.HEADS, cfg.HID, cfg.HC1, cfg.OUT

    # block-diagonal per-head attention matrices: al = h @ A1s
    A1s = np.zeros((HC1, H), np.float32)
    A1d = np.zeros((HC1, H), np.float32)
    for h in range(H):
        A1s[h * HID:(h + 1) * HID, h] = a1_src[h]
        A1d[h * HID:(h + 1) * HID, h] = a1_dst[h]

    common = {
        "W1": np.ascontiguousarray(W1),
        "W1T": np.ascontiguousarray(W1.T),
        "A1s": A1s, "A1d": A1d,
        "B1rep": np.tile(b1[None, :], (128, 1)).astype(np.float32),
        "W2": np.ascontiguousarray(W2),
        "W2T": np.ascontiguousarray(W2.T),
        "a2s": np.ascontiguousarray(a2_src.reshape(OUT, 1)),
        "a2d": np.ascontiguousarray(a2_dst.reshape(OUT, 1)),
        "B2rep": np.tile(b2[None, :], (128, 1)).astype(np.float32),
    }
    in_maps = []
    for r in range(cfg.R):
        m = dict(common)
        xp = np.zeros((cfg.T * 128, x.shape[1]), np.float32)
        xp[:cfg.NPR] = x[sch.perm[r]]
        m["xT"] = np.ascontiguousarray(xp.T)
        m["idxs"] = np.ascontiguousarray(sch.idx16[r])
        in_maps.append(m)
    return in_maps


def run(cfg: Cfg, inputs: dict, trace: bool = False):
    edge_index = np.asarray(inputs["edge_index"])
    loops = np.arange(cfg.N, dtype=edge_index.dtype)
    src = np.concatenate([edge_index[0], loops]).astype(np.int64)
    dst = np.concatenate([edge_index[1], loops]).astype(np.int64)

    sch = build_schedule(cfg, src, dst)
    nc = build_program(cfg, sch)
    in_maps = _host_inputs(cfg, sch, inputs)
    res = bass_utils.run_bass_kernel_spmd(
        nc, in_maps, core_ids=list(range(cfg.R)), trace=trace)
    out = np.empty((cfg.N, cfg.OUT), np.float32)
    for r in range(cfg.R):
        o = res.results[r]["out"]
        out[sch.perm[r]] = o[:cfg.NPR]
    return out, res


def kernel(**inputs) -> np.ndarray:
    cfg = Cfg()
    out, _ = run(cfg, inputs)
    return out


if __name__ == "__main__":
    import reference
    inputs = {k: np.asarray(v) for k, v in reference.setup_inputs().items()}
    out = kernel(**inputs)
    exp = np.asarray(reference.reference(**reference.setup_inputs()))
    err = np.abs(out - exp).max() / (np.abs(exp).max() + 1e-12)
    print("rel err:", err)
